# revision 11
# baseline (speedup 1.0000x reference)
"""Trainium2 Bass kernel for Gaussian-upsampling attention (duration/range
BiLSTM predictors + Gaussian score attention), data-parallel over batch
across 8 NeuronCores.

kernel(**inputs) takes the full unsharded inputs (as in reference
setup_inputs) and returns (durations [B,N,1] f32, att [B,T,D] f32).

Numerics: all matmuls run as bf16 hi/lo-split pieces (weights AND moving
operands split into bf16 high + bf16 residual; three cross products
accumulate in fp32 PSUM), giving ~1.5e-5 effective relative error, except
the small cumsum (triangular) matmul which is plain fp32. Activations
(sigmoid/tanh/exp) use the ACT LUTs (~1e-6).
"""
import sys

for _p in ("/opt/trn_rl_repo", "/root/.axon_site", "/root/.axon_site/_ro/trn_rl_repo"):
    if _p not in sys.path:
        sys.path.append(_p)

import numpy as np
import ml_dtypes

import concourse.bass as bass
import concourse.mybir as mybir
import concourse.tile as tile
import bass_rust
from concourse.bass_utils import run_bass_kernel_spmd

F32 = mybir.dt.float32
BF16 = mybir.dt.bfloat16
AF = mybir.ActivationFunctionType
OP = mybir.AluOpType

# problem shapes (hardcoded per spec)
B, N, D, H, T = 32, 512, 256, 256, 2048
NCORES = 8
BC = B // NCORES          # batches per core = 4
KT = H // 128             # K-tiles of hidden dim = 2
MC = 4 * H // 128         # gate chunks = 8
EPS = 1e-6
XQ = 128                  # xg staging chunk (tokens)

_BUILD_CACHE = {}
LAST_RES = None


# ---------------------------------------------------------------- wait split
def _split_excess_waits(nc, cap=1):
    """walrus in this env rejects >cap sync-waits on an instruction; hoist
    excess waits onto preceding same-engine NOPs."""
    n_created = 0
    for f in nc.m.functions:
        for blk in f.blocks:
            insts = blk.instructions
            i = 0
            while i < len(insts):
                inst = insts[i]
                si = inst.sync_info
                waits = list(si.on_wait) if si is not None else []
                if len(waits) > cap:
                    keep = waits[:cap]
                    extra = waits[cap:]
                    inst.sync_info = bass_rust.SyncInfo(
                        on_wait=keep, on_update=list(si.on_update))
                    pos = i
                    for j in range(0, len(extra), cap):
                        chunk = extra[j:j + cap]
                        nop = mybir.InstNoOp(
                            name=f"I-waitsplit-{n_created}", ins=[], outs=[])
                        nop.engine = inst.engine
                        nop.sync_info = bass_rust.SyncInfo(
                            on_wait=chunk, on_update=[])
                        nc.register_instruction(nop)
                        insts.insert(pos, nop)
                        pos += 1
                        i += 1
                        n_created += 1
                i += 1
    return n_created


# ---------------------------------------------------------------- build
def build_nc(cfg):
    n = cfg["N"]; t_out = cfg["T"]
    nch = n // 128
    tch = t_out // 128
    nq = n // XQ
    nc = bass.Bass()

    dram = {}

    def din(name, shape, dtype=F32):
        dram[name] = nc.declare_dram_parameter(name, list(shape), dtype,
                                               isOutput=False)
        return dram[name]

    def dout(name, shape, dtype=F32):
        dram[name] = nc.declare_dram_parameter(name, list(shape), dtype,
                                               isOutput=True)
        return dram[name]

    din("xT_hi", [128, KT, BC, n], BF16)
    din("xT_lo", [128, KT, BC, n], BF16)
    din("xhat_hi", [128, nch, BC, D + 1], BF16)
    din("xhat_lo", [128, nch, BC, D + 1], BF16)
    for ls in ("dur", "rng"):
        for dr in ("f", "b"):
            for pc in ("hi", "lo"):
                din(f"whh_{ls}_{dr}_{pc}", [128, KT, 4 * H], BF16)
                din(f"wih_{ls}_{dr}_{pc}", [128, KT, 4 * H], BF16)
    for dr in ("f", "b"):
        din(f"brow_dur_{dr}", [2, 4 * H], BF16)    # [bias_hi; bias_lo]
        din(f"wdA_rng_{dr}", [98, 4 * H], BF16)    # [w_d_hi; bias_hi] @ 32b
        din(f"wdB_rng_{dr}", [98, 4 * H], BF16)    # [w_d_lo; bias_lo]
        din(f"wdC_rng_{dr}", [98, 4 * H], BF16)    # [w_d_hi; 0]
    for ls in ("dur", "rng"):
        for pc in ("hi", "lo"):
            din(f"pwT_{ls}_{pc}", [128, 2 * KT, 1], BF16)
    din("cons", [128, 4], F32)          # cols: dur_pb, rng_pb, eps, 0
    din("scanrhs_init", [128, n], BF16)  # ones rows at partitions 32b+1
    din("ltri", [128, 2, 128], F32)     # [ones block, (tril-0.5I)^T block]
    din("pe", [128, tch, D], F32)

    dout("dur_out", [128, nch, BC], F32)
    dout("att_out", [BC, tch, 128, D], F32)
    if cfg.get("DEBUG"):
        dout("dbg_xgf", [128, MC, BC, XQ], F32)
        dout("dbg_histfhi", [128, KT, BC, n], BF16)
        dout("dbg_histflo", [128, KT, BC, n], BF16)
        dout("dbg_histbhi", [128, KT, BC, n], BF16)
        dout("dbg_gs1", [128, 2, MC, BC], F32)

    with tile.TileContext(nc) as tc:
        with tc.tile_pool(name="glob", bufs=1) as glob:
            cons_t = glob.tile([128, 4], F32)
            nc.sync.dma_start(cons_t[:], dram["cons"][:])
            ltri = glob.tile([128, 2, 128], F32)
            nc.sync.dma_start(ltri[:], dram["ltri"][:])
            pwT = {}
            for ls in ("dur", "rng"):
                for pc in ("hi", "lo"):
                    pwT[ls, pc] = glob.tile([128, 2 * KT, 1], BF16, name=f"pwT_{ls}_{pc}",
                                            tag=f"pwT_{ls}_{pc}")
                    nc.sync.dma_start(pwT[ls, pc][:], dram[f"pwT_{ls}_{pc}"][:])
            dT = glob.tile([128, nch, BC], F32)
            negcT = glob.tile([128, nch, BC], F32)
            rT = glob.tile([128, nch, BC], F32)
            nir2T = glob.tile([128, nch, BC], F32)
            # rng-xg rhs rows: [d_hi;1] and [d_lo;0] at partitions {32b,32b+1}
            scanrhs_hi = glob.tile([128, n], BF16)
            scanrhs_lo = glob.tile([128, n], BF16)
            nc.sync.dma_start(scanrhs_hi[:], dram["scanrhs_init"][:])
            nc.vector.memset(scanrhs_lo[:], 0.0)
            dfree = glob.tile([128, n], F32)
            zero_h = glob.tile([128, KT, BC], BF16)
            nc.vector.memset(zero_h[:], 0.0)
            ones2 = glob.tile([2, n], BF16)
            nc.vector.memset(ones2[:], 1.0)

            # ================= LSTM phases =================
            for ls in ("dur", "rng"):
                with tc.tile_pool(name=f"ph{ls}", bufs=1) as php:
                    whh = {}; wih = {}
                    for dr in ("f", "b"):
                        for pc in ("hi", "lo"):
                            whh[dr, pc] = php.tile([128, KT, 4 * H], BF16, name=f"whh{dr}{pc}",
                                                   tag=f"whh{dr}{pc}")
                            nc.sync.dma_start(whh[dr, pc][:],
                                              dram[f"whh_{ls}_{dr}_{pc}"][:])
                            wih[dr, pc] = php.tile([128, KT, 4 * H], BF16, name=f"wih{dr}{pc}",
                                                   tag=f"wih{dr}{pc}")
                            nc.sync.dma_start(wih[dr, pc][:],
                                              dram[f"wih_{ls}_{dr}_{pc}"][:])
                    xT_hi = php.tile([128, KT, BC, n], BF16, tag="xthi")
                    xT_lo = php.tile([128, KT, BC, n], BF16, tag="xtlo")
                    nc.sync.dma_start(xT_hi[:], dram["xT_hi"][:])
                    nc.sync.dma_start(xT_lo[:], dram["xT_lo"][:])
                    wrows = {}
                    for dr in ("f", "b"):
                        if ls == "dur":
                            br = php.tile([2, 4 * H], BF16, tag=f"br{dr}")
                            nc.sync.dma_start(br[:], dram[f"brow_dur_{dr}"][:])
                            wrows[dr] = br
                        else:
                            rows = []
                            for nm in ("wdA", "wdB", "wdC"):
                                wt = php.tile([98, 4 * H], BF16, name=f"{nm}{dr}",
                                              tag=f"{nm}{dr}")
                                nc.sync.dma_start(
                                    wt[:], dram[f"{nm}_rng_{dr}"][:])
                                rows.append(wt)
                            wrows[dr] = rows

                    hist = {}
                    for dr in ("f", "b"):
                        for pc in ("hi", "lo"):
                            hist[dr, pc] = php.tile(
                                [128, KT, BC, n], BF16,
                                name=f"hist{dr}{pc}", tag=f"hist{dr}{pc}")
                    c_t = php.tile([128, 2, KT, BC], F32, tag="c")
                    nc.vector.memset(c_t[:], 0.0)

                    # ---- xg staging GEMM (one XQ-token chunk, one dir) ----
                    def emit_xg_chunk(dr, q, xgpool, psum):
                        xt = xgpool.tile([128, MC, BC, XQ], F32,
                                         tag=f"xg{dr}")
                        tsl = slice(q * XQ, (q + 1) * XQ)
                        for m in range(MC):
                            msl = slice(m * 128, (m + 1) * 128)
                            for bi in range(BC):
                                po = psum.tile([128, XQ], F32, tag="xp")
                                nmm = 3 * KT + (1 if ls == "dur" else 3)
                                cnt = 0
                                for wp, xp in ((wih[dr, "hi"], xT_hi),
                                               (wih[dr, "lo"], xT_hi),
                                               (wih[dr, "hi"], xT_lo)):
                                    for k in range(KT):
                                        cnt += 1
                                        nc.tensor.matmul(
                                            po[:], wp[:, k, msl],
                                            xp[:, k, bi, tsl],
                                            start=(cnt == 1),
                                            stop=(cnt == nmm))
                                if ls == "dur":
                                    cnt += 1
                                    nc.tensor.matmul(
                                        po[:], wrows[dr][0:2, msl],
                                        ones2[0:2, tsl],
                                        start=False, stop=(cnt == nmm))
                                else:
                                    pb = 32 * bi
                                    for wt, rr in (
                                            (wrows[dr][0], scanrhs_hi),
                                            (wrows[dr][1], scanrhs_hi),
                                            (wrows[dr][2], scanrhs_lo)):
                                        cnt += 1
                                        nc.tensor.matmul(
                                            po[:],
                                            wt[pb:pb + 2, msl],
                                            rr[pb:pb + 2, tsl],
                                            start=False, stop=(cnt == nmm),
                                            tile_position=(pb, 0))
                                nc.vector.tensor_copy(xt[:, m, bi, :], po[:])
                        return xt

                    # ---- the scan ----
                    nc.enter_named_scope(f"scan_{ls}", False)
                    with tc.tile_pool(name=f"xgq{ls}", bufs=2) as xgpool, \
                         tc.tile_pool(name=f"xgp{ls}", bufs=3,
                                      space="PSUM") as xpsum, \
                         tc.tile_pool(name=f"scan{ls}", bufs=3) as scp, \
                         tc.tile_pool(name=f"scanp{ls}", bufs=2,
                                      space="PSUM") as spsum:
                        xq_cur = {"f": emit_xg_chunk("f", 0, xgpool, xpsum),
                                  "b": emit_xg_chunk("b", nq - 1, xgpool,
                                                     xpsum)}
                        if cfg.get("DEBUG") and ls == "dur":
                            nc.sync.dma_start(dram["dbg_xgf"][:],
                                              xq_cur["f"][:])
                        xq_nxt = {}
                        for t in range(n):
                            qw = t // XQ
                            if t % XQ == 8 and qw + 1 < nq:
                                xq_nxt["f"] = emit_xg_chunk(
                                    "f", qw + 1, xgpool, xpsum)
                                xq_nxt["b"] = emit_xg_chunk(
                                    "b", nq - 2 - qw, xgpool, xpsum)
                            if t % XQ == 0 and t > 0:
                                xq_cur = dict(xq_nxt)
                            toks = {"f": t, "b": n - 1 - t}
                            gs = scp.tile([128, 2, MC, BC], F32, tag="gs")
                            for di, dr in enumerate(("f", "b")):
                                tok = toks[dr]
                                po = spsum.tile([128, MC * BC], F32,
                                                tag=f"g{dr}")
                                if t == 0:
                                    pieces = [(whh[dr, "hi"], zero_h, None),
                                              (whh[dr, "lo"], zero_h, None),
                                              (whh[dr, "hi"], zero_h, None)]
                                else:
                                    prev = tok + (1 if dr == "b" else -1)
                                    pieces = [
                                        (whh[dr, "hi"], hist[dr, "hi"], prev),
                                        (whh[dr, "lo"], hist[dr, "hi"], prev),
                                        (whh[dr, "hi"], hist[dr, "lo"], prev)]
                                for m in range(MC):
                                    cnt = 0
                                    for wp, hp, prev in pieces:
                                        for k in range(KT):
                                            cnt += 1
                                            rhs = (hp[:, k, :] if prev is None
                                                   else hp[:, k, :, prev])
                                            nc.tensor.matmul(
                                                po[:, m * BC:(m + 1) * BC],
                                                wp[:, k,
                                                   m * 128:(m + 1) * 128],
                                                rhs,
                                                start=(cnt == 1),
                                                stop=(cnt == 3 * KT))
                                xgt = xq_cur[dr]
                                off = tok % XQ
                                nc.vector.scalar_tensor_tensor(
                                    gs[:, di],
                                    po[:].rearrange("p (m b) -> p m b", b=BC),
                                    0.0, xgt[:, :, :, off],
                                    op0=OP.add, op1=OP.add)
                            if cfg.get("DEBUG") and ls == "dur" and t == 1:
                                nc.sync.dma_start(dram["dbg_gs1"][:], gs[:])
                            sg = scp.tile([128, 2, MC, BC], F32, tag="sg")
                            nc.scalar.activation(sg[:, :, 0:6, :],
                                                 gs[:, :, 0:6, :], AF.Sigmoid)
                            nc.scalar.activation(sg[:, :, 6:8, :],
                                                 gs[:, :, 6:8, :], AF.Tanh)
                            m1 = scp.tile([128, 2, KT, BC], F32, tag="m1")
                            nc.vector.tensor_mul(m1[:], sg[:, :, 0:2, :],
                                                 sg[:, :, 6:8, :])
                            nc.vector.tensor_mul(c_t[:], sg[:, :, 2:4, :],
                                                 c_t[:])
                            nc.vector.tensor_add(c_t[:], c_t[:], m1[:])
                            th = scp.tile([128, 2, KT, BC], F32, tag="th")
                            nc.scalar.activation(th[:], c_t[:], AF.Tanh)
                            hfp = scp.tile([128, 2, KT, BC], F32, tag="hfp")
                            nc.vector.tensor_mul(hfp[:], sg[:, :, 4:6, :],
                                                 th[:])
                            for di, dr in enumerate(("f", "b")):
                                tok = toks[dr]
                                nc.vector.tensor_copy(
                                    hist[dr, "hi"][:, :, :, tok], hfp[:, di])
                                nc.vector.tensor_sub(
                                    hist[dr, "lo"][:, :, :, tok],
                                    hfp[:, di], hist[dr, "hi"][:, :, :, tok])

                    if cfg.get("DEBUG") and ls == "dur":
                        nc.sync.dma_start(dram["dbg_histfhi"][:],
                                          hist["f", "hi"][:])
                        nc.sync.dma_start(dram["dbg_histflo"][:],
                                          hist["f", "lo"][:])
                        nc.sync.dma_start(dram["dbg_histbhi"][:],
                                          hist["b", "hi"][:])
                    nc._state.pop_named_scope(f"scan_{ls}")
                    # ---- projection to d^T (dur) or r^T (rng), += pb ----
                    proj = dT if ls == "dur" else rT
                    pbi = 0 if ls == "dur" else 1
                    with tc.tile_pool(name=f"pj{ls}", bufs=4,
                                      space="PSUM") as ppsum:
                        for bi in range(BC):
                            for q in range(nch):
                                po = ppsum.tile([128, 1], F32, tag="pp")
                                qsl = slice(q * 128, (q + 1) * 128)
                                cnt = 0
                                pieces = (("hi", "hi"), ("lo", "hi"),
                                          ("hi", "lo"))
                                nmm = len(pieces) * 2 * KT
                                for hp, wp in pieces:
                                    for di, dr in enumerate(("f", "b")):
                                        for k in range(KT):
                                            cnt += 1
                                            nc.tensor.matmul(
                                                po[:],
                                                hist[dr, hp][:, k, bi, qsl],
                                                pwT[ls, wp][:, di * KT + k,
                                                            :],
                                                start=(cnt == 1),
                                                stop=(cnt == nmm))
                                nc.vector.tensor_scalar(
                                    proj[:, q, bi:bi + 1], po[:],
                                    cons_t[:, pbi:pbi + 1], None, op0=OP.add)

                    if ls == "dur":
                        nc.sync.dma_start(dram["dur_out"][:], dT[:])
                        # d rows (free layout at partitions 32b) for rng xg
                        with tc.tile_pool(name="dfp", bufs=4,
                                          space="PSUM") as dpsum:
                            for bi in range(BC):
                                po = dpsum.tile([128, n], F32, tag="df")
                                pb = 32 * bi
                                cnt = 0
                                pieces = (("hi", "hi"), ("lo", "hi"),
                                          ("hi", "lo"))
                                nmm = len(pieces) * 2 * KT
                                for hp, wp in pieces:
                                    for di, dr in enumerate(("f", "b")):
                                        for k in range(KT):
                                            cnt += 1
                                            nc.tensor.matmul(
                                                po[pb:pb + 1, :],
                                                pwT[ls, wp][:, di * KT + k,
                                                            :],
                                                hist[dr, hp][:, k, bi, :],
                                                start=(cnt == 1),
                                                stop=(cnt == nmm),
                                                tile_position=(0, pb))
                                nc.vector.tensor_scalar(
                                    dfree[pb:pb + 1, :], po[pb:pb + 1, :],
                                    cons_t[pb:pb + 1, 0:1], None, op0=OP.add)
                                nc.vector.tensor_copy(
                                    scanrhs_hi[pb:pb + 1, :],
                                    dfree[pb:pb + 1, :])
                                nc.vector.tensor_sub(
                                    scanrhs_lo[pb:pb + 1, :],
                                    dfree[pb:pb + 1, :],
                                    scanrhs_hi[pb:pb + 1, :])
                        # centers: -c^T = -(L - 0.5I) @ d
                        with tc.tile_pool(name="ctr", bufs=4,
                                          space="PSUM") as cpsum:
                            for q in range(nch):
                                po = cpsum.tile([128, BC], F32, tag="cp")
                                for s in range(q + 1):
                                    nc.tensor.matmul(
                                        po[:],
                                        ltri[:, 1 if s == q else 0, :],
                                        dT[:, s, :],
                                        start=(s == 0), stop=(s == q))
                                nc.vector.tensor_scalar(
                                    negcT[:, q, :], po[:], -1.0, None,
                                    op0=OP.mult)

            # nir2T = -(1/r)^2
            nc.vector.reciprocal(rT[:], rT[:])
            nc.vector.scalar_tensor_tensor(nir2T[:], rT[:], -1.0, rT[:],
                                           op0=OP.mult, op1=OP.mult)

            # ================= scores + bmm =================
            nc.enter_named_scope("scores", False)
            with tc.tile_pool(name="sc", bufs=1) as scg:
                pe_t = scg.tile([128, tch, D], F32)
                nc.sync.dma_start(pe_t[:], dram["pe"][:])
                xhat_hi = scg.tile([128, nch, BC, D + 1], BF16)
                xhat_lo = scg.tile([128, nch, BC, D + 1], BF16)
                nc.sync.dma_start(xhat_hi[:], dram["xhat_hi"][:])
                nc.sync.dma_start(xhat_lo[:], dram["xhat_lo"][:])
                tiota = scg.tile([128, t_out], F32)
                nc.gpsimd.iota(tiota[:], pattern=[[1, t_out]], base=0,
                               channel_multiplier=0,
                               allow_small_or_imprecise_dtypes=True)
                with tc.tile_pool(name="wbuf", bufs=2) as wbp, \
                     tc.tile_pool(name="wtmp", bufs=2) as wtp, \
                     tc.tile_pool(name="scp", bufs=4, space="PSUM") as apsum:
                    for bi in range(BC):
                        whi = wbp.tile([128, nch, t_out], BF16, tag="whi")
                        wlo = wbp.tile([128, nch, t_out], BF16, tag="wlo")
                        for q in range(nch):
                            u = wtp.tile([128, t_out], F32, tag="u")
                            nc.vector.tensor_scalar(
                                u[:], tiota[:], negcT[:, q, bi:bi + 1], None,
                                op0=OP.add)
                            u2 = wtp.tile([128, t_out], F32, tag="u2")
                            nc.vector.scalar_tensor_tensor(
                                u2[:], u[:], nir2T[:, q, bi:bi + 1], u[:],
                                op0=OP.mult, op1=OP.mult)
                            wf = wtp.tile([128, t_out], F32, tag="wf")
                            nc.scalar.activation(wf[:], u2[:], AF.Exp)
                            nc.scalar.copy(whi[:, q, :], wf[:])
                            nc.vector.tensor_sub(wlo[:, q, :], wf[:],
                                                 whi[:, q, :])
                        nmm = 3 * nch
                        for j in range(tch):
                            po = apsum.tile([128, D + 1], F32, tag="ap")
                            jsl = slice(j * 128, (j + 1) * 128)
                            cnt = 0
                            for wp, xp in ((whi, xhat_hi), (wlo, xhat_hi),
                                           (whi, xhat_lo)):
                                for q in range(nch):
                                    cnt += 1
                                    nc.tensor.matmul(
                                        po[:], wp[:, q, jsl],
                                        xp[:, q, bi, :],
                                        start=(cnt == 1), stop=(cnt == nmm))
                            srec = wtp.tile([128, 1], F32, tag="srec")
                            nc.vector.tensor_scalar(
                                srec[:], po[:, D:D + 1], cons_t[:, 2:3],
                                None, op0=OP.add)
                            nc.vector.reciprocal(srec[:], srec[:])
                            att_t = wtp.tile([128, D], F32, tag="att")
                            nc.vector.scalar_tensor_tensor(
                                att_t[:], po[:, 0:D], srec[:, 0:1],
                                pe_t[:, j, :], op0=OP.mult, op1=OP.add)
                            nc.sync.dma_start(
                                dram["att_out"][bi, j, :, :], att_t[:])
            nc._state.pop_named_scope("scores")

    _split_excess_waits(nc)
    return nc, dram


# ---------------------------------------------------------------- host prep
def _bfsplit(a):
    hi = a.astype(ml_dtypes.bfloat16)
    lo = (a - hi.astype(np.float32)).astype(ml_dtypes.bfloat16)
    return hi, lo


def _perm4h():
    """gate order [i,f,g,o] (torch) -> chunk blocks [i,f,o,g]."""
    i = np.arange(H)
    return np.concatenate([i, H + i, 3 * H + i, 2 * H + i])


def _prep_kxm(Wt):
    """[K_total, M] -> [128, KT, M] (K on partitions)."""
    ktot, m = Wt.shape
    return Wt.reshape(ktot // 128, 128, m).transpose(1, 0, 2).copy()


def kernel(embeddings, input_lengths, T_out,
           dur_Wf, dur_Uf, dur_bf, dur_Wb, dur_Ub, dur_bb, dur_pw, dur_pb,
           rng_Wf, rng_Uf, rng_bf, rng_Wb, rng_Ub, rng_bb, rng_pw, rng_pb,
           pe, _cfg=None, _trace=False):
    global LAST_RES
    cfg = {"N": N, "T": T}
    if _cfg:
        cfg.update(_cfg)
    n, t_out = cfg["N"], cfg["T"]
    nch = n // 128
    tch = t_out // 128

    emb = np.asarray(embeddings, dtype=np.float32)
    perm = _perm4h()

    key = ("nc", n, t_out)
    if key not in _BUILD_CACHE:
        _BUILD_CACHE[key] = build_nc(cfg)
    nc, dram = _BUILD_CACHE[key]

    rep = {}
    for ls, Wf_, Uf_, bf_, Wb_, Ub_, bb_ in (
            ("dur", dur_Wf, dur_Uf, dur_bf, dur_Wb, dur_Ub, dur_bb),
            ("rng", rng_Wf, rng_Uf, rng_bf, rng_Wb, rng_Ub, rng_bb)):
        for dr, W_, U_, b_ in (("f", Wf_, Uf_, bf_), ("b", Wb_, Ub_, bb_)):
            W_ = np.asarray(W_, np.float32)
            U_ = np.asarray(U_, np.float32)
            whh = _prep_kxm(U_.T[:, perm])
            hi, lo = _bfsplit(whh)
            rep[f"whh_{ls}_{dr}_hi"], rep[f"whh_{ls}_{dr}_lo"] = hi, lo
            wx = _prep_kxm(W_[:, :D].T[:, perm])
            hi, lo = _bfsplit(wx)
            rep[f"wih_{ls}_{dr}_hi"], rep[f"wih_{ls}_{dr}_lo"] = hi, lo
            bp = np.asarray(b_, np.float32)[perm]
            bhi, blo = _bfsplit(bp)
            if ls == "dur":
                rep[f"brow_dur_{dr}"] = np.stack([bhi, blo])
            else:
                wd = np.asarray(W_, np.float32)[perm, D]
                wdhi, wdlo = _bfsplit(wd)
                za = np.zeros((98, 4 * H), ml_dtypes.bfloat16)
                zb = np.zeros((98, 4 * H), ml_dtypes.bfloat16)
                zc = np.zeros((98, 4 * H), ml_dtypes.bfloat16)
                for bi in range(BC):
                    za[32 * bi] = wdhi; za[32 * bi + 1] = bhi
                    zb[32 * bi] = wdlo; zb[32 * bi + 1] = blo
                    zc[32 * bi] = wdhi
                rep[f"wdA_rng_{dr}"] = za
                rep[f"wdB_rng_{dr}"] = zb
                rep[f"wdC_rng_{dr}"] = zc
    for ls, pw_ in (("dur", dur_pw), ("rng", rng_pw)):
        pw_ = np.asarray(pw_, np.float32).reshape(2 * H)
        pwT = pw_.reshape(2 * KT, 128).T.reshape(128, 2 * KT, 1).copy()
        hi, lo = _bfsplit(pwT)
        rep[f"pwT_{ls}_hi"], rep[f"pwT_{ls}_lo"] = hi, lo
    consrow = np.array([float(np.asarray(dur_pb).reshape(-1)[0]),
                        float(np.asarray(rng_pb).reshape(-1)[0]),
                        EPS, 0.0], np.float32)
    rep["cons"] = np.tile(consrow[None, :], (128, 1))
    sri = np.zeros((128, n), ml_dtypes.bfloat16)
    for bi in range(BC):
        sri[32 * bi + 1] = 1.0
    rep["scanrhs_init"] = sri
    ones_blk = np.ones((128, 128), np.float32)
    tri_blk = (np.tril(np.ones((128, 128), np.float32))
               - 0.5 * np.eye(128, dtype=np.float32))
    rep["ltri"] = np.stack([ones_blk, tri_blk.T.copy()], axis=1)
    pe_ = np.asarray(pe, np.float32)[:t_out]
    rep["pe"] = pe_.reshape(tch, 128, D).transpose(1, 0, 2).copy()

    in_maps = []
    for c in range(NCORES):
        m = dict(rep)
        ec = emb[c * BC:(c + 1) * BC, :n]             # [BC, n, D]
        xT = ec.transpose(2, 0, 1).reshape(KT, 128, BC, n)\
            .transpose(1, 0, 2, 3).copy()             # [128, KT, BC, n]
        hi, lo = _bfsplit(xT)
        m["xT_hi"], m["xT_lo"] = hi, lo
        xhat = np.concatenate(
            [ec, np.ones((BC, n, 1), np.float32)], axis=2)
        xhat = xhat.reshape(BC, nch, 128, D + 1).transpose(2, 1, 0, 3).copy()
        hi, lo = _bfsplit(xhat)
        m["xhat_hi"], m["xhat_lo"] = hi, lo
        in_maps.append(m)

    if _trace:
        import trnprof
        trnprof.install()
    res = run_bass_kernel_spmd(nc, in_maps, core_ids=list(range(NCORES)),
                               trace=_trace)
    LAST_RES = res

    durations = np.zeros((B, n, 1), np.float32)
    att = np.zeros((B, t_out, D), np.float32)
    for c in range(NCORES):
        r = res.results[c]
        durations[c * BC:(c + 1) * BC, :, 0] = \
            r["dur_out"].transpose(2, 1, 0).reshape(BC, n)
        att[c * BC:(c + 1) * BC] = r["att_out"].reshape(BC, t_out, D)
    return durations, att


# revision 12
# speedup vs baseline: 1.0167x; 1.0167x over previous
"""Trainium2 Bass kernel for Gaussian-upsampling attention (duration/range
BiLSTM predictors + Gaussian score attention), data-parallel over batch
across 8 NeuronCores.

kernel(**inputs) takes the full unsharded inputs (as in reference
setup_inputs) and returns (durations [B,N,1] f32, att [B,T,D] f32).

Numerics: all matmuls run as bf16 hi/lo-split pieces (weights AND moving
operands split into bf16 high + bf16 residual; three cross products
accumulate in fp32 PSUM), giving ~1.5e-5 effective relative error, except
the small cumsum (triangular) matmul which is plain fp32. Activations
(sigmoid/tanh/exp) use the ACT LUTs (~1e-6).
"""
import sys

for _p in ("/opt/trn_rl_repo", "/root/.axon_site", "/root/.axon_site/_ro/trn_rl_repo"):
    if _p not in sys.path:
        sys.path.append(_p)

import numpy as np
import ml_dtypes

import concourse.bass as bass
import concourse.mybir as mybir
import concourse.tile as tile
import bass_rust
from concourse.bass_utils import run_bass_kernel_spmd

F32 = mybir.dt.float32
BF16 = mybir.dt.bfloat16
AF = mybir.ActivationFunctionType
OP = mybir.AluOpType

# problem shapes (hardcoded per spec)
B, N, D, H, T = 32, 512, 256, 256, 2048
NCORES = 8
BC = B // NCORES          # batches per core = 4
KT = H // 128             # K-tiles of hidden dim = 2
MC = 4 * H // 128         # gate chunks = 8
EPS = 1e-6
XQ = 128                  # xg staging chunk (tokens)

_BUILD_CACHE = {}
LAST_RES = None


# ---------------------------------------------------------------- wait split
def _split_excess_waits(nc, cap=1):
    """walrus in this env rejects >cap sync-waits on an instruction; hoist
    excess waits onto preceding same-engine NOPs."""
    n_created = 0
    for f in nc.m.functions:
        for blk in f.blocks:
            insts = blk.instructions
            i = 0
            while i < len(insts):
                inst = insts[i]
                si = inst.sync_info
                waits = list(si.on_wait) if si is not None else []
                if len(waits) > cap:
                    keep = waits[:cap]
                    extra = waits[cap:]
                    inst.sync_info = bass_rust.SyncInfo(
                        on_wait=keep, on_update=list(si.on_update))
                    pos = i
                    for j in range(0, len(extra), cap):
                        chunk = extra[j:j + cap]
                        nop = mybir.InstNoOp(
                            name=f"I-waitsplit-{n_created}", ins=[], outs=[])
                        nop.engine = inst.engine
                        nop.sync_info = bass_rust.SyncInfo(
                            on_wait=chunk, on_update=[])
                        nc.register_instruction(nop)
                        insts.insert(pos, nop)
                        pos += 1
                        i += 1
                        n_created += 1
                i += 1
    return n_created


# ---------------------------------------------------------------- build
def build_nc(cfg):
    n = cfg["N"]; t_out = cfg["T"]
    nch = n // 128
    tch = t_out // 128
    nq = n // XQ
    nc = bass.Bass()

    dram = {}

    def din(name, shape, dtype=F32):
        dram[name] = nc.declare_dram_parameter(name, list(shape), dtype,
                                               isOutput=False)
        return dram[name]

    def dout(name, shape, dtype=F32):
        dram[name] = nc.declare_dram_parameter(name, list(shape), dtype,
                                               isOutput=True)
        return dram[name]

    din("xT_hi", [128, KT, BC, n], BF16)
    din("xT_lo", [128, KT, BC, n], BF16)
    din("xhat_hi", [128, nch, BC, D + 1], BF16)
    din("xhat_lo", [128, nch, BC, D + 1], BF16)
    for ls in ("dur", "rng"):
        for dr in ("f", "b"):
            for pc in ("hi", "lo"):
                din(f"whh_{ls}_{dr}_{pc}", [128, KT, 4 * H], BF16)
                din(f"wih_{ls}_{dr}_{pc}", [128, KT, 4 * H], BF16)
    for dr in ("f", "b"):
        din(f"brow_dur_{dr}", [2, 4 * H], BF16)    # [bias_hi; bias_lo]
        din(f"wdA_rng_{dr}", [98, 4 * H], BF16)    # [w_d_hi; bias_hi] @ 32b
        din(f"wdB_rng_{dr}", [98, 4 * H], BF16)    # [w_d_lo; bias_lo]
        din(f"wdC_rng_{dr}", [98, 4 * H], BF16)    # [w_d_hi; 0]
    for ls in ("dur", "rng"):
        for pc in ("hi", "lo"):
            din(f"pwT_{ls}_{pc}", [128, 2 * KT, 1], BF16)
    din("cons", [128, 4], F32)          # cols: dur_pb, rng_pb, eps, 0
    din("scanrhs_init", [128, n], BF16)  # ones rows at partitions 32b+1
    din("ltri", [128, 2, 128], F32)     # [ones block, (tril-0.5I)^T block]
    din("pe", [128, tch, D], F32)

    dout("dur_out", [128, nch, BC], F32)
    dout("att_out", [BC, tch, 128, D], F32)
    if cfg.get("DEBUG"):
        dout("dbg_xgf", [128, MC, BC, XQ], F32)
        dout("dbg_histfhi", [128, KT, BC, n], BF16)
        dout("dbg_histflo", [128, KT, BC, n], BF16)
        dout("dbg_histbhi", [128, KT, BC, n], BF16)
        dout("dbg_gs1", [128, 2, MC, BC], F32)

    with tile.TileContext(nc) as tc:
        with tc.tile_pool(name="glob", bufs=1) as glob:
            cons_t = glob.tile([128, 4], F32)
            nc.sync.dma_start(cons_t[:], dram["cons"][:])
            ltri = glob.tile([128, 2, 128], F32)
            nc.sync.dma_start(ltri[:], dram["ltri"][:])
            pwT = {}
            for ls in ("dur", "rng"):
                for pc in ("hi", "lo"):
                    pwT[ls, pc] = glob.tile([128, 2 * KT, 1], BF16, name=f"pwT_{ls}_{pc}",
                                            tag=f"pwT_{ls}_{pc}")
                    nc.sync.dma_start(pwT[ls, pc][:], dram[f"pwT_{ls}_{pc}"][:])
            dT = glob.tile([128, nch, BC], F32)
            negcT = glob.tile([128, nch, BC], F32)
            rT = glob.tile([128, nch, BC], F32)
            nir2T = glob.tile([128, nch, BC], F32)
            # rng-xg rhs rows: [d_hi;1] and [d_lo;0] at partitions {32b,32b+1}
            scanrhs_hi = glob.tile([128, n], BF16)
            scanrhs_lo = glob.tile([128, n], BF16)
            nc.sync.dma_start(scanrhs_hi[:], dram["scanrhs_init"][:])
            nc.vector.memset(scanrhs_lo[:], 0.0)
            dfree = glob.tile([128, n], F32)
            zero_h = glob.tile([128, KT, BC], BF16)
            nc.vector.memset(zero_h[:], 0.0)
            ones2 = glob.tile([2, n], BF16)
            nc.vector.memset(ones2[:], 1.0)

            # ================= LSTM phases =================
            for ls in ("dur", "rng"):
                with tc.tile_pool(name=f"ph{ls}", bufs=1) as php:
                    whh = {}; wih = {}
                    for dr in ("f", "b"):
                        for pc in ("hi", "lo"):
                            whh[dr, pc] = php.tile([128, KT, 4 * H], BF16, name=f"whh{dr}{pc}",
                                                   tag=f"whh{dr}{pc}")
                            nc.sync.dma_start(whh[dr, pc][:],
                                              dram[f"whh_{ls}_{dr}_{pc}"][:])
                            wih[dr, pc] = php.tile([128, KT, 4 * H], BF16, name=f"wih{dr}{pc}",
                                                   tag=f"wih{dr}{pc}")
                            nc.sync.dma_start(wih[dr, pc][:],
                                              dram[f"wih_{ls}_{dr}_{pc}"][:])
                    xT_hi = php.tile([128, KT, BC, n], BF16, tag="xthi")
                    xT_lo = php.tile([128, KT, BC, n], BF16, tag="xtlo")
                    nc.sync.dma_start(xT_hi[:], dram["xT_hi"][:])
                    nc.sync.dma_start(xT_lo[:], dram["xT_lo"][:])
                    wrows = {}
                    for dr in ("f", "b"):
                        if ls == "dur":
                            br = php.tile([2, 4 * H], BF16, tag=f"br{dr}")
                            nc.sync.dma_start(br[:], dram[f"brow_dur_{dr}"][:])
                            wrows[dr] = br
                        else:
                            rows = []
                            for nm in ("wdA", "wdB", "wdC"):
                                wt = php.tile([98, 4 * H], BF16, name=f"{nm}{dr}",
                                              tag=f"{nm}{dr}")
                                nc.sync.dma_start(
                                    wt[:], dram[f"{nm}_rng_{dr}"][:])
                                rows.append(wt)
                            wrows[dr] = rows

                    hist = {}
                    for dr in ("f", "b"):
                        for pc in ("hi", "lo"):
                            hist[dr, pc] = php.tile(
                                [128, KT, BC, n], BF16,
                                name=f"hist{dr}{pc}", tag=f"hist{dr}{pc}")
                    c_t = {}
                    for dr in ("f", "b"):
                        c_t[dr] = php.tile([128, KT, BC], F32, name=f"c{dr}",
                                           tag=f"c{dr}")
                        nc.vector.memset(c_t[dr][:], 0.0)

                    # ---- xg staging GEMM (one XQ-token chunk, one dir) ----
                    def emit_xg_chunk(dr, q, xgpool, psum):
                        xt = xgpool.tile([128, MC, BC, XQ], F32,
                                         tag=f"xg{dr}")
                        tsl = slice(q * XQ, (q + 1) * XQ)
                        for m in range(MC):
                            msl = slice(m * 128, (m + 1) * 128)
                            for bi in range(BC):
                                po = psum.tile([128, XQ], F32, tag="xp")
                                nmm = 3 * KT + (1 if ls == "dur" else 3)
                                cnt = 0
                                for wp, xp in ((wih[dr, "hi"], xT_hi),
                                               (wih[dr, "lo"], xT_hi),
                                               (wih[dr, "hi"], xT_lo)):
                                    for k in range(KT):
                                        cnt += 1
                                        nc.tensor.matmul(
                                            po[:], wp[:, k, msl],
                                            xp[:, k, bi, tsl],
                                            start=(cnt == 1),
                                            stop=(cnt == nmm))
                                if ls == "dur":
                                    cnt += 1
                                    nc.tensor.matmul(
                                        po[:], wrows[dr][0:2, msl],
                                        ones2[0:2, tsl],
                                        start=False, stop=(cnt == nmm))
                                else:
                                    pb = 32 * bi
                                    for wt, rr in (
                                            (wrows[dr][0], scanrhs_hi),
                                            (wrows[dr][1], scanrhs_hi),
                                            (wrows[dr][2], scanrhs_lo)):
                                        cnt += 1
                                        nc.tensor.matmul(
                                            po[:],
                                            wt[pb:pb + 2, msl],
                                            rr[pb:pb + 2, tsl],
                                            start=False, stop=(cnt == nmm),
                                            tile_position=(pb, 0))
                                nc.vector.tensor_copy(xt[:, m, bi, :], po[:])
                        return xt

                    # ---- the scan ----
                    nc.enter_named_scope(f"scan_{ls}", False)
                    with tc.tile_pool(name=f"xgq{ls}", bufs=2) as xgpool, \
                         tc.tile_pool(name=f"xgp{ls}", bufs=3,
                                      space="PSUM") as xpsum, \
                         tc.tile_pool(name=f"scan{ls}", bufs=3) as scp, \
                         tc.tile_pool(name=f"scanp{ls}", bufs=2,
                                      space="PSUM") as spsum:
                        xq_cur = {"f": emit_xg_chunk("f", 0, xgpool, xpsum),
                                  "b": emit_xg_chunk("b", nq - 1, xgpool,
                                                     xpsum)}
                        if cfg.get("DEBUG") and ls == "dur":
                            nc.sync.dma_start(dram["dbg_xgf"][:],
                                              xq_cur["f"][:])
                        xq_nxt = {}
                        for t in range(n):
                            qw = t // XQ
                            if t % XQ == 8 and qw + 1 < nq:
                                xq_nxt["f"] = emit_xg_chunk(
                                    "f", qw + 1, xgpool, xpsum)
                                xq_nxt["b"] = emit_xg_chunk(
                                    "b", nq - 2 - qw, xgpool, xpsum)
                            if t % XQ == 0 and t > 0:
                                xq_cur = dict(xq_nxt)
                            toks = {"f": t, "b": n - 1 - t}
                            for di, dr in enumerate(("f", "b")):
                                tok = toks[dr]
                                po = spsum.tile([128, MC * BC], F32,
                                                tag=f"g{dr}")
                                if t == 0:
                                    pieces = [(whh[dr, "hi"], zero_h, None),
                                              (whh[dr, "lo"], zero_h, None),
                                              (whh[dr, "hi"], zero_h, None)]
                                else:
                                    prev = tok + (1 if dr == "b" else -1)
                                    pieces = [
                                        (whh[dr, "hi"], hist[dr, "hi"], prev),
                                        (whh[dr, "lo"], hist[dr, "hi"], prev),
                                        (whh[dr, "hi"], hist[dr, "lo"], prev)]
                                for m in range(MC):
                                    cnt = 0
                                    for wp, hp, prev in pieces:
                                        for k in range(KT):
                                            cnt += 1
                                            rhs = (hp[:, k, :] if prev is None
                                                   else hp[:, k, :, prev])
                                            nc.tensor.matmul(
                                                po[:, m * BC:(m + 1) * BC],
                                                wp[:, k,
                                                   m * 128:(m + 1) * 128],
                                                rhs,
                                                start=(cnt == 1),
                                                stop=(cnt == 3 * KT))
                                xgt = xq_cur[dr]
                                off = tok % XQ
                                gs = scp.tile([128, MC, BC], F32,
                                              name=f"gs{dr}", tag=f"gs{dr}")
                                nc.vector.scalar_tensor_tensor(
                                    gs[:],
                                    po[:].rearrange("p (m b) -> p m b", b=BC),
                                    0.0, xgt[:, :, :, off],
                                    op0=OP.add, op1=OP.add)
                                sg = scp.tile([128, MC, BC], F32,
                                              name=f"sg{dr}", tag=f"sg{dr}")
                                nc.scalar.activation(sg[:, 0:6, :],
                                                     gs[:, 0:6, :],
                                                     AF.Sigmoid)
                                nc.scalar.activation(sg[:, 6:8, :],
                                                     gs[:, 6:8, :], AF.Tanh)
                                m1 = scp.tile([128, KT, BC], F32,
                                              name=f"m1{dr}", tag=f"m1{dr}")
                                nc.vector.tensor_mul(m1[:], sg[:, 0:2, :],
                                                     sg[:, 6:8, :])
                                nc.vector.tensor_mul(c_t[dr][:],
                                                     sg[:, 2:4, :],
                                                     c_t[dr][:])
                                nc.vector.tensor_add(c_t[dr][:], c_t[dr][:],
                                                     m1[:])
                                th = scp.tile([128, KT, BC], F32,
                                              name=f"th{dr}", tag=f"th{dr}")
                                nc.scalar.activation(th[:], c_t[dr][:],
                                                     AF.Tanh)
                                hfp = scp.tile([128, KT, BC], F32,
                                               name=f"hfp{dr}",
                                               tag=f"hfp{dr}")
                                nc.vector.tensor_mul(hfp[:], sg[:, 4:6, :],
                                                     th[:])
                                nc.vector.tensor_copy(
                                    hist[dr, "hi"][:, :, :, tok], hfp[:])
                                nc.vector.tensor_sub(
                                    hist[dr, "lo"][:, :, :, tok],
                                    hfp[:], hist[dr, "hi"][:, :, :, tok])

                    if cfg.get("DEBUG") and ls == "dur":
                        nc.sync.dma_start(dram["dbg_histfhi"][:],
                                          hist["f", "hi"][:])
                        nc.sync.dma_start(dram["dbg_histflo"][:],
                                          hist["f", "lo"][:])
                        nc.sync.dma_start(dram["dbg_histbhi"][:],
                                          hist["b", "hi"][:])
                    nc._state.pop_named_scope(f"scan_{ls}")
                    # ---- projection to d^T (dur) or r^T (rng), += pb ----
                    proj = dT if ls == "dur" else rT
                    pbi = 0 if ls == "dur" else 1
                    with tc.tile_pool(name=f"pj{ls}", bufs=4,
                                      space="PSUM") as ppsum:
                        for bi in range(BC):
                            for q in range(nch):
                                po = ppsum.tile([128, 1], F32, tag="pp")
                                qsl = slice(q * 128, (q + 1) * 128)
                                cnt = 0
                                pieces = (("hi", "hi"), ("lo", "hi"),
                                          ("hi", "lo"))
                                nmm = len(pieces) * 2 * KT
                                for hp, wp in pieces:
                                    for di, dr in enumerate(("f", "b")):
                                        for k in range(KT):
                                            cnt += 1
                                            nc.tensor.matmul(
                                                po[:],
                                                hist[dr, hp][:, k, bi, qsl],
                                                pwT[ls, wp][:, di * KT + k,
                                                            :],
                                                start=(cnt == 1),
                                                stop=(cnt == nmm))
                                nc.vector.tensor_scalar(
                                    proj[:, q, bi:bi + 1], po[:],
                                    cons_t[:, pbi:pbi + 1], None, op0=OP.add)

                    if ls == "dur":
                        nc.sync.dma_start(dram["dur_out"][:], dT[:])
                        # d rows (free layout at partitions 32b) for rng xg
                        with tc.tile_pool(name="dfp", bufs=4,
                                          space="PSUM") as dpsum:
                            for bi in range(BC):
                                po = dpsum.tile([128, n], F32, tag="df")
                                pb = 32 * bi
                                cnt = 0
                                pieces = (("hi", "hi"), ("lo", "hi"),
                                          ("hi", "lo"))
                                nmm = len(pieces) * 2 * KT
                                for hp, wp in pieces:
                                    for di, dr in enumerate(("f", "b")):
                                        for k in range(KT):
                                            cnt += 1
                                            nc.tensor.matmul(
                                                po[pb:pb + 1, :],
                                                pwT[ls, wp][:, di * KT + k,
                                                            :],
                                                hist[dr, hp][:, k, bi, :],
                                                start=(cnt == 1),
                                                stop=(cnt == nmm),
                                                tile_position=(0, pb))
                                nc.vector.tensor_scalar(
                                    dfree[pb:pb + 1, :], po[pb:pb + 1, :],
                                    cons_t[pb:pb + 1, 0:1], None, op0=OP.add)
                                nc.vector.tensor_copy(
                                    scanrhs_hi[pb:pb + 1, :],
                                    dfree[pb:pb + 1, :])
                                nc.vector.tensor_sub(
                                    scanrhs_lo[pb:pb + 1, :],
                                    dfree[pb:pb + 1, :],
                                    scanrhs_hi[pb:pb + 1, :])
                        # centers: -c^T = -(L - 0.5I) @ d
                        with tc.tile_pool(name="ctr", bufs=4,
                                          space="PSUM") as cpsum:
                            for q in range(nch):
                                po = cpsum.tile([128, BC], F32, tag="cp")
                                for s in range(q + 1):
                                    nc.tensor.matmul(
                                        po[:],
                                        ltri[:, 1 if s == q else 0, :],
                                        dT[:, s, :],
                                        start=(s == 0), stop=(s == q))
                                nc.vector.tensor_scalar(
                                    negcT[:, q, :], po[:], -1.0, None,
                                    op0=OP.mult)

            # nir2T = -(1/r)^2
            nc.vector.reciprocal(rT[:], rT[:])
            nc.vector.scalar_tensor_tensor(nir2T[:], rT[:], -1.0, rT[:],
                                           op0=OP.mult, op1=OP.mult)

            # ================= scores + bmm =================
            nc.enter_named_scope("scores", False)
            with tc.tile_pool(name="sc", bufs=1) as scg:
                pe_t = scg.tile([128, tch, D], F32)
                nc.sync.dma_start(pe_t[:], dram["pe"][:])
                xhat_hi = scg.tile([128, nch, BC, D + 1], BF16)
                xhat_lo = scg.tile([128, nch, BC, D + 1], BF16)
                nc.sync.dma_start(xhat_hi[:], dram["xhat_hi"][:])
                nc.sync.dma_start(xhat_lo[:], dram["xhat_lo"][:])
                tiota = scg.tile([128, t_out], F32)
                nc.gpsimd.iota(tiota[:], pattern=[[1, t_out]], base=0,
                               channel_multiplier=0,
                               allow_small_or_imprecise_dtypes=True)
                with tc.tile_pool(name="wbuf", bufs=2) as wbp, \
                     tc.tile_pool(name="wtmp", bufs=2) as wtp, \
                     tc.tile_pool(name="scp", bufs=4, space="PSUM") as apsum:
                    for bi in range(BC):
                        whi = wbp.tile([128, nch, t_out], BF16, tag="whi")
                        wlo = wbp.tile([128, nch, t_out], BF16, tag="wlo")
                        for q in range(nch):
                            u = wtp.tile([128, t_out], F32, tag="u")
                            nc.vector.tensor_scalar(
                                u[:], tiota[:], negcT[:, q, bi:bi + 1], None,
                                op0=OP.add)
                            u2 = wtp.tile([128, t_out], F32, tag="u2")
                            nc.vector.scalar_tensor_tensor(
                                u2[:], u[:], nir2T[:, q, bi:bi + 1], u[:],
                                op0=OP.mult, op1=OP.mult)
                            wf = wtp.tile([128, t_out], F32, tag="wf")
                            nc.scalar.activation(wf[:], u2[:], AF.Exp)
                            nc.scalar.copy(whi[:, q, :], wf[:])
                            nc.vector.tensor_sub(wlo[:, q, :], wf[:],
                                                 whi[:, q, :])
                        nmm = 3 * nch
                        for j in range(tch):
                            po = apsum.tile([128, D + 1], F32, tag="ap")
                            jsl = slice(j * 128, (j + 1) * 128)
                            cnt = 0
                            for wp, xp in ((whi, xhat_hi), (wlo, xhat_hi),
                                           (whi, xhat_lo)):
                                for q in range(nch):
                                    cnt += 1
                                    nc.tensor.matmul(
                                        po[:], wp[:, q, jsl],
                                        xp[:, q, bi, :],
                                        start=(cnt == 1), stop=(cnt == nmm))
                            srec = wtp.tile([128, 1], F32, tag="srec")
                            nc.vector.tensor_scalar(
                                srec[:], po[:, D:D + 1], cons_t[:, 2:3],
                                None, op0=OP.add)
                            nc.vector.reciprocal(srec[:], srec[:])
                            att_t = wtp.tile([128, D], F32, tag="att")
                            nc.vector.scalar_tensor_tensor(
                                att_t[:], po[:, 0:D], srec[:, 0:1],
                                pe_t[:, j, :], op0=OP.mult, op1=OP.add)
                            nc.sync.dma_start(
                                dram["att_out"][bi, j, :, :], att_t[:])
            nc._state.pop_named_scope("scores")

    _split_excess_waits(nc)
    return nc, dram


# ---------------------------------------------------------------- host prep
def _bfsplit(a):
    hi = a.astype(ml_dtypes.bfloat16)
    lo = (a - hi.astype(np.float32)).astype(ml_dtypes.bfloat16)
    return hi, lo


def _perm4h():
    """gate order [i,f,g,o] (torch) -> chunk blocks [i,f,o,g]."""
    i = np.arange(H)
    return np.concatenate([i, H + i, 3 * H + i, 2 * H + i])


def _prep_kxm(Wt):
    """[K_total, M] -> [128, KT, M] (K on partitions)."""
    ktot, m = Wt.shape
    return Wt.reshape(ktot // 128, 128, m).transpose(1, 0, 2).copy()


def kernel(embeddings, input_lengths, T_out,
           dur_Wf, dur_Uf, dur_bf, dur_Wb, dur_Ub, dur_bb, dur_pw, dur_pb,
           rng_Wf, rng_Uf, rng_bf, rng_Wb, rng_Ub, rng_bb, rng_pw, rng_pb,
           pe, _cfg=None, _trace=False):
    global LAST_RES
    cfg = {"N": N, "T": T}
    if _cfg:
        cfg.update(_cfg)
    n, t_out = cfg["N"], cfg["T"]
    nch = n // 128
    tch = t_out // 128

    emb = np.asarray(embeddings, dtype=np.float32)
    perm = _perm4h()

    key = ("nc", n, t_out)
    if key not in _BUILD_CACHE:
        _BUILD_CACHE[key] = build_nc(cfg)
    nc, dram = _BUILD_CACHE[key]

    rep = {}
    for ls, Wf_, Uf_, bf_, Wb_, Ub_, bb_ in (
            ("dur", dur_Wf, dur_Uf, dur_bf, dur_Wb, dur_Ub, dur_bb),
            ("rng", rng_Wf, rng_Uf, rng_bf, rng_Wb, rng_Ub, rng_bb)):
        for dr, W_, U_, b_ in (("f", Wf_, Uf_, bf_), ("b", Wb_, Ub_, bb_)):
            W_ = np.asarray(W_, np.float32)
            U_ = np.asarray(U_, np.float32)
            whh = _prep_kxm(U_.T[:, perm])
            hi, lo = _bfsplit(whh)
            rep[f"whh_{ls}_{dr}_hi"], rep[f"whh_{ls}_{dr}_lo"] = hi, lo
            wx = _prep_kxm(W_[:, :D].T[:, perm])
            hi, lo = _bfsplit(wx)
            rep[f"wih_{ls}_{dr}_hi"], rep[f"wih_{ls}_{dr}_lo"] = hi, lo
            bp = np.asarray(b_, np.float32)[perm]
            bhi, blo = _bfsplit(bp)
            if ls == "dur":
                rep[f"brow_dur_{dr}"] = np.stack([bhi, blo])
            else:
                wd = np.asarray(W_, np.float32)[perm, D]
                wdhi, wdlo = _bfsplit(wd)
                za = np.zeros((98, 4 * H), ml_dtypes.bfloat16)
                zb = np.zeros((98, 4 * H), ml_dtypes.bfloat16)
                zc = np.zeros((98, 4 * H), ml_dtypes.bfloat16)
                for bi in range(BC):
                    za[32 * bi] = wdhi; za[32 * bi + 1] = bhi
                    zb[32 * bi] = wdlo; zb[32 * bi + 1] = blo
                    zc[32 * bi] = wdhi
                rep[f"wdA_rng_{dr}"] = za
                rep[f"wdB_rng_{dr}"] = zb
                rep[f"wdC_rng_{dr}"] = zc
    for ls, pw_ in (("dur", dur_pw), ("rng", rng_pw)):
        pw_ = np.asarray(pw_, np.float32).reshape(2 * H)
        pwT = pw_.reshape(2 * KT, 128).T.reshape(128, 2 * KT, 1).copy()
        hi, lo = _bfsplit(pwT)
        rep[f"pwT_{ls}_hi"], rep[f"pwT_{ls}_lo"] = hi, lo
    consrow = np.array([float(np.asarray(dur_pb).reshape(-1)[0]),
                        float(np.asarray(rng_pb).reshape(-1)[0]),
                        EPS, 0.0], np.float32)
    rep["cons"] = np.tile(consrow[None, :], (128, 1))
    sri = np.zeros((128, n), ml_dtypes.bfloat16)
    for bi in range(BC):
        sri[32 * bi + 1] = 1.0
    rep["scanrhs_init"] = sri
    ones_blk = np.ones((128, 128), np.float32)
    tri_blk = (np.tril(np.ones((128, 128), np.float32))
               - 0.5 * np.eye(128, dtype=np.float32))
    rep["ltri"] = np.stack([ones_blk, tri_blk.T.copy()], axis=1)
    pe_ = np.asarray(pe, np.float32)[:t_out]
    rep["pe"] = pe_.reshape(tch, 128, D).transpose(1, 0, 2).copy()

    in_maps = []
    for c in range(NCORES):
        m = dict(rep)
        ec = emb[c * BC:(c + 1) * BC, :n]             # [BC, n, D]
        xT = ec.transpose(2, 0, 1).reshape(KT, 128, BC, n)\
            .transpose(1, 0, 2, 3).copy()             # [128, KT, BC, n]
        hi, lo = _bfsplit(xT)
        m["xT_hi"], m["xT_lo"] = hi, lo
        xhat = np.concatenate(
            [ec, np.ones((BC, n, 1), np.float32)], axis=2)
        xhat = xhat.reshape(BC, nch, 128, D + 1).transpose(2, 1, 0, 3).copy()
        hi, lo = _bfsplit(xhat)
        m["xhat_hi"], m["xhat_lo"] = hi, lo
        in_maps.append(m)

    if _trace:
        import trnprof
        trnprof.install()
    res = run_bass_kernel_spmd(nc, in_maps, core_ids=list(range(NCORES)),
                               trace=_trace)
    LAST_RES = res

    durations = np.zeros((B, n, 1), np.float32)
    att = np.zeros((B, t_out, D), np.float32)
    for c in range(NCORES):
        r = res.results[c]
        durations[c * BC:(c + 1) * BC, :, 0] = \
            r["dur_out"].transpose(2, 1, 0).reshape(BC, n)
        att[c * BC:(c + 1) * BC] = r["att_out"].reshape(BC, t_out, D)
    return durations, att


# revision 13
# speedup vs baseline: 1.0482x; 1.0310x over previous
"""Trainium2 Bass kernel for Gaussian-upsampling attention (duration/range
BiLSTM predictors + Gaussian score attention), data-parallel over batch
across 8 NeuronCores.

kernel(**inputs) takes the full unsharded inputs (as in reference
setup_inputs) and returns (durations [B,N,1] f32, att [B,T,D] f32).

Numerics: all matmuls run as bf16 hi/lo-split pieces (weights AND moving
operands split into bf16 high + bf16 residual; three cross products
accumulate in fp32 PSUM), giving ~1.5e-5 effective relative error, except
the small cumsum (triangular) matmul which is plain fp32. Activations
(sigmoid/tanh/exp) use the ACT LUTs (~1e-6).
"""
import sys

for _p in ("/opt/trn_rl_repo", "/root/.axon_site", "/root/.axon_site/_ro/trn_rl_repo"):
    if _p not in sys.path:
        sys.path.append(_p)

import numpy as np
import ml_dtypes

import concourse.bass as bass
import concourse.mybir as mybir
import concourse.tile as tile
import bass_rust
from concourse.bass_utils import run_bass_kernel_spmd

F32 = mybir.dt.float32
BF16 = mybir.dt.bfloat16
AF = mybir.ActivationFunctionType
OP = mybir.AluOpType

# problem shapes (hardcoded per spec)
B, N, D, H, T = 32, 512, 256, 256, 2048
NCORES = 8
BC = B // NCORES          # batches per core = 4
KT = H // 128             # K-tiles of hidden dim = 2
MC = 4 * H // 128         # gate chunks = 8
EPS = 1e-6
XQ = 128                  # xg staging chunk (tokens)

_BUILD_CACHE = {}
LAST_RES = None


# ---------------------------------------------------------------- wait split
def _split_excess_waits(nc, cap=1):
    """walrus in this env rejects >cap sync-waits on an instruction; hoist
    excess waits onto preceding same-engine NOPs."""
    n_created = 0
    for f in nc.m.functions:
        for blk in f.blocks:
            insts = blk.instructions
            i = 0
            while i < len(insts):
                inst = insts[i]
                si = inst.sync_info
                waits = list(si.on_wait) if si is not None else []
                if len(waits) > cap:
                    keep = waits[:cap]
                    extra = waits[cap:]
                    inst.sync_info = bass_rust.SyncInfo(
                        on_wait=keep, on_update=list(si.on_update))
                    pos = i
                    for j in range(0, len(extra), cap):
                        chunk = extra[j:j + cap]
                        nop = mybir.InstNoOp(
                            name=f"I-waitsplit-{n_created}", ins=[], outs=[])
                        nop.engine = inst.engine
                        nop.sync_info = bass_rust.SyncInfo(
                            on_wait=chunk, on_update=[])
                        nc.register_instruction(nop)
                        insts.insert(pos, nop)
                        pos += 1
                        i += 1
                        n_created += 1
                i += 1
    return n_created


# ---------------------------------------------------------------- build
def build_nc(cfg):
    n = cfg["N"]; t_out = cfg["T"]
    nch = n // 128
    tch = t_out // 128
    nq = n // XQ
    nc = bass.Bass()

    dram = {}

    def din(name, shape, dtype=F32):
        dram[name] = nc.declare_dram_parameter(name, list(shape), dtype,
                                               isOutput=False)
        return dram[name]

    def dout(name, shape, dtype=F32):
        dram[name] = nc.declare_dram_parameter(name, list(shape), dtype,
                                               isOutput=True)
        return dram[name]

    din("xT_hi", [128, KT, BC, n], BF16)
    din("xT_lo", [128, KT, BC, n], BF16)
    din("xhat_hi", [128, nch, BC, D + 1], BF16)
    din("xhat_lo", [128, nch, BC, D + 1], BF16)
    for ls in ("dur", "rng"):
        for dr in ("f", "b"):
            for pc in ("hi", "lo"):
                din(f"whh_{ls}_{dr}_{pc}", [128, KT, 4 * H], BF16)
                din(f"wih_{ls}_{dr}_{pc}", [128, KT, 4 * H], BF16)
    for dr in ("f", "b"):
        din(f"brow_dur_{dr}", [2, 4 * H], BF16)    # [bias_hi; bias_lo]
        din(f"wdA_rng_{dr}", [98, 4 * H], BF16)    # [w_d_hi; bias_hi] @ 32b
        din(f"wdB_rng_{dr}", [98, 4 * H], BF16)    # [w_d_lo; bias_lo]
        din(f"wdC_rng_{dr}", [98, 4 * H], BF16)    # [w_d_hi; 0]
    for ls in ("dur", "rng"):
        for pc in ("hi", "lo"):
            din(f"pwT_{ls}_{pc}", [128, 2 * KT, 1], BF16)
    din("cons", [128, 4], F32)          # cols: dur_pb, rng_pb, eps, 0
    din("scanrhs_init", [128, n], BF16)  # ones rows at partitions 32b+1
    din("ltri", [128, 2, 128], F32)     # [ones block, (tril-0.5I)^T block]
    din("pe", [128, tch, D], F32)

    dout("dur_out", [128, nch, BC], F32)
    dout("att_out", [BC, tch, 128, D], F32)
    if cfg.get("DEBUG"):
        dout("dbg_xgf", [128, MC, BC, XQ], F32)
        dout("dbg_histfhi", [128, KT, BC, n], BF16)
        dout("dbg_histflo", [128, KT, BC, n], BF16)
        dout("dbg_histbhi", [128, KT, BC, n], BF16)
        dout("dbg_gs1", [128, 2, MC, BC], F32)

    with tile.TileContext(nc) as tc:
        with tc.tile_pool(name="glob", bufs=1) as glob:
            cons_t = glob.tile([128, 4], F32)
            nc.sync.dma_start(cons_t[:], dram["cons"][:])
            ltri = glob.tile([128, 2, 128], F32)
            nc.sync.dma_start(ltri[:], dram["ltri"][:])
            pwT = {}
            for ls in ("dur", "rng"):
                for pc in ("hi", "lo"):
                    pwT[ls, pc] = glob.tile([128, 2 * KT, 1], BF16, name=f"pwT_{ls}_{pc}",
                                            tag=f"pwT_{ls}_{pc}")
                    nc.sync.dma_start(pwT[ls, pc][:], dram[f"pwT_{ls}_{pc}"][:])
            dT = glob.tile([128, nch, BC], F32)
            negcT = glob.tile([128, nch, BC], F32)
            rT = glob.tile([128, nch, BC], F32)
            nir2T = glob.tile([128, nch, BC], F32)
            # rng-xg rhs rows: [d_hi;1] and [d_lo;0] at partitions {32b,32b+1}
            scanrhs_hi = glob.tile([128, n], BF16)
            scanrhs_lo = glob.tile([128, n], BF16)
            nc.sync.dma_start(scanrhs_hi[:], dram["scanrhs_init"][:])
            nc.vector.memset(scanrhs_lo[:], 0.0)
            dfree = glob.tile([128, n], F32)
            zero_h = glob.tile([128, KT, BC], BF16)
            nc.vector.memset(zero_h[:], 0.0)
            ones2 = glob.tile([2, n], BF16)
            nc.vector.memset(ones2[:], 1.0)

            # ================= LSTM phases =================
            for ls in ("dur", "rng"):
                with tc.tile_pool(name=f"ph{ls}", bufs=1) as php:
                    whh = {}; wih = {}
                    for dr in ("f", "b"):
                        for pc in ("hi", "lo"):
                            whh[dr, pc] = php.tile([128, KT, 4 * H], BF16, name=f"whh{dr}{pc}",
                                                   tag=f"whh{dr}{pc}")
                            nc.sync.dma_start(whh[dr, pc][:],
                                              dram[f"whh_{ls}_{dr}_{pc}"][:])
                            wih[dr, pc] = php.tile([128, KT, 4 * H], BF16, name=f"wih{dr}{pc}",
                                                   tag=f"wih{dr}{pc}")
                            nc.sync.dma_start(wih[dr, pc][:],
                                              dram[f"wih_{ls}_{dr}_{pc}"][:])
                    xT_hi = php.tile([128, KT, BC, n], BF16, tag="xthi")
                    xT_lo = php.tile([128, KT, BC, n], BF16, tag="xtlo")
                    nc.sync.dma_start(xT_hi[:], dram["xT_hi"][:])
                    nc.sync.dma_start(xT_lo[:], dram["xT_lo"][:])
                    wrows = {}
                    for dr in ("f", "b"):
                        if ls == "dur":
                            br = php.tile([2, 4 * H], BF16, tag=f"br{dr}")
                            nc.sync.dma_start(br[:], dram[f"brow_dur_{dr}"][:])
                            wrows[dr] = br
                        else:
                            rows = []
                            for nm in ("wdA", "wdB", "wdC"):
                                wt = php.tile([98, 4 * H], BF16, name=f"{nm}{dr}",
                                              tag=f"{nm}{dr}")
                                nc.sync.dma_start(
                                    wt[:], dram[f"{nm}_rng_{dr}"][:])
                                rows.append(wt)
                            wrows[dr] = rows

                    hist = {}
                    for dr in ("f", "b"):
                        for pc in ("hi", "lo"):
                            hist[dr, pc] = php.tile(
                                [128, KT, BC, n], BF16,
                                name=f"hist{dr}{pc}", tag=f"hist{dr}{pc}")
                    c_t = {}
                    for dr in ("f", "b"):
                        c_t[dr] = php.tile([128, KT, BC], F32, name=f"c{dr}",
                                           tag=f"c{dr}")
                        nc.vector.memset(c_t[dr][:], 0.0)

                    # ---- xg staging GEMM (one XQ-token chunk, one dir).
                    # Returns (tile, [block emitters]) so blocks can be
                    # spread between scan steps to fill PE stalls. ----
                    def make_xg_chunk(dr, q, xgpool, psum):
                        xt = xgpool.tile([128, MC, BC, XQ], F32,
                                         name=f"xg{dr}", tag=f"xg{dr}")
                        tsl = slice(q * XQ, (q + 1) * XQ)

                        def mk(m, bi):
                            def emit():
                                msl = slice(m * 128, (m + 1) * 128)
                                po = psum.tile([128, XQ], F32, name="xp",
                                               tag="xp")
                                nmm = 3 * KT + (1 if ls == "dur" else 3)
                                cnt = 0
                                for wp, xp in ((wih[dr, "hi"], xT_hi),
                                               (wih[dr, "lo"], xT_hi),
                                               (wih[dr, "hi"], xT_lo)):
                                    for k in range(KT):
                                        cnt += 1
                                        nc.tensor.matmul(
                                            po[:], wp[:, k, msl],
                                            xp[:, k, bi, tsl],
                                            start=(cnt == 1),
                                            stop=(cnt == nmm))
                                if ls == "dur":
                                    cnt += 1
                                    nc.tensor.matmul(
                                        po[:], wrows[dr][0:2, msl],
                                        ones2[0:2, tsl],
                                        start=False, stop=(cnt == nmm))
                                else:
                                    pb = 32 * bi
                                    for wt, rr in (
                                            (wrows[dr][0], scanrhs_hi),
                                            (wrows[dr][1], scanrhs_hi),
                                            (wrows[dr][2], scanrhs_lo)):
                                        cnt += 1
                                        nc.tensor.matmul(
                                            po[:],
                                            wt[pb:pb + 2, msl],
                                            rr[pb:pb + 2, tsl],
                                            start=False, stop=(cnt == nmm),
                                            tile_position=(pb, 0))
                                nc.vector.tensor_copy(xt[:, m, bi, :], po[:])
                            return emit

                        blocks = [mk(m, bi) for m in range(MC)
                                  for bi in range(BC)]
                        return xt, blocks

                    def emit_xg_chunk(dr, q, xgpool, psum):
                        xt, blocks = make_xg_chunk(dr, q, xgpool, psum)
                        for blk in blocks:
                            blk()
                        return xt

                    # ---- the scan ----
                    nc.enter_named_scope(f"scan_{ls}", False)
                    with tc.tile_pool(name=f"xgq{ls}", bufs=2) as xgpool, \
                         tc.tile_pool(name=f"xgp{ls}", bufs=3,
                                      space="PSUM") as xpsum, \
                         tc.tile_pool(name=f"scan{ls}", bufs=3) as scp, \
                         tc.tile_pool(name=f"scanp{ls}", bufs=2,
                                      space="PSUM") as spsum:
                        xq_cur = {"f": emit_xg_chunk("f", 0, xgpool, xpsum),
                                  "b": emit_xg_chunk("b", nq - 1, xgpool,
                                                     xpsum)}
                        if cfg.get("DEBUG") and ls == "dur":
                            nc.sync.dma_start(dram["dbg_xgf"][:],
                                              xq_cur["f"][:])
                        xq_nxt = {}
                        pending_blocks = []
                        for t in range(n):
                            qw = t // XQ
                            if t % XQ == 8 and qw + 1 < nq:
                                xq_nxt["f"], bl_f = make_xg_chunk(
                                    "f", qw + 1, xgpool, xpsum)
                                xq_nxt["b"], bl_b = make_xg_chunk(
                                    "b", nq - 2 - qw, xgpool, xpsum)
                                # interleave f/b blocks
                                pending_blocks = [blk for pair in
                                                  zip(bl_f, bl_b)
                                                  for blk in pair]
                            if pending_blocks and t % XQ >= 8:
                                pending_blocks.pop(0)()
                            if t % XQ == 0 and t > 0:
                                for blk in pending_blocks:
                                    blk()
                                pending_blocks = []
                                xq_cur = dict(xq_nxt)
                            toks = {"f": t, "b": n - 1 - t}
                            for di, dr in enumerate(("f", "b")):
                                tok = toks[dr]
                                po = spsum.tile([128, MC * BC], F32,
                                                tag=f"g{dr}")
                                if t == 0:
                                    pieces = [(whh[dr, "hi"], zero_h, None),
                                              (whh[dr, "lo"], zero_h, None),
                                              (whh[dr, "hi"], zero_h, None)]
                                else:
                                    prev = tok + (1 if dr == "b" else -1)
                                    pieces = [
                                        (whh[dr, "hi"], hist[dr, "hi"], prev),
                                        (whh[dr, "lo"], hist[dr, "hi"], prev),
                                        (whh[dr, "hi"], hist[dr, "lo"], prev)]
                                for m in range(MC):
                                    cnt = 0
                                    for wp, hp, prev in pieces:
                                        for k in range(KT):
                                            cnt += 1
                                            rhs = (hp[:, k, :] if prev is None
                                                   else hp[:, k, :, prev])
                                            nc.tensor.matmul(
                                                po[:, m * BC:(m + 1) * BC],
                                                wp[:, k,
                                                   m * 128:(m + 1) * 128],
                                                rhs,
                                                start=(cnt == 1),
                                                stop=(cnt == 3 * KT))
                                xgt = xq_cur[dr]
                                off = tok % XQ
                                gs = scp.tile([128, MC, BC], F32,
                                              name=f"gs{dr}", tag=f"gs{dr}")
                                nc.vector.scalar_tensor_tensor(
                                    gs[:],
                                    po[:].rearrange("p (m b) -> p m b", b=BC),
                                    0.0, xgt[:, :, :, off],
                                    op0=OP.add, op1=OP.add)
                                sg = scp.tile([128, MC, BC], F32,
                                              name=f"sg{dr}", tag=f"sg{dr}")
                                nc.scalar.activation(sg[:, 0:6, :],
                                                     gs[:, 0:6, :],
                                                     AF.Sigmoid)
                                nc.scalar.activation(sg[:, 6:8, :],
                                                     gs[:, 6:8, :], AF.Tanh)
                                m1 = scp.tile([128, KT, BC], F32,
                                              name=f"m1{dr}", tag=f"m1{dr}")
                                nc.vector.tensor_mul(m1[:], sg[:, 0:2, :],
                                                     sg[:, 6:8, :])
                                nc.vector.tensor_mul(c_t[dr][:],
                                                     sg[:, 2:4, :],
                                                     c_t[dr][:])
                                nc.vector.tensor_add(c_t[dr][:], c_t[dr][:],
                                                     m1[:])
                                th = scp.tile([128, KT, BC], F32,
                                              name=f"th{dr}", tag=f"th{dr}")
                                nc.scalar.activation(th[:], c_t[dr][:],
                                                     AF.Tanh)
                                hfp = scp.tile([128, KT, BC], F32,
                                               name=f"hfp{dr}",
                                               tag=f"hfp{dr}")
                                nc.vector.tensor_mul(hfp[:], sg[:, 4:6, :],
                                                     th[:])
                                nc.vector.tensor_copy(
                                    hist[dr, "hi"][:, :, :, tok], hfp[:])
                                nc.vector.tensor_sub(
                                    hist[dr, "lo"][:, :, :, tok],
                                    hfp[:], hist[dr, "hi"][:, :, :, tok])

                    if cfg.get("DEBUG") and ls == "dur":
                        nc.sync.dma_start(dram["dbg_histfhi"][:],
                                          hist["f", "hi"][:])
                        nc.sync.dma_start(dram["dbg_histflo"][:],
                                          hist["f", "lo"][:])
                        nc.sync.dma_start(dram["dbg_histbhi"][:],
                                          hist["b", "hi"][:])
                    nc._state.pop_named_scope(f"scan_{ls}")
                    # ---- projection to d^T (dur) or r^T (rng), += pb ----
                    proj = dT if ls == "dur" else rT
                    pbi = 0 if ls == "dur" else 1
                    with tc.tile_pool(name=f"pj{ls}", bufs=4,
                                      space="PSUM") as ppsum:
                        for bi in range(BC):
                            for q in range(nch):
                                po = ppsum.tile([128, 1], F32, tag="pp")
                                qsl = slice(q * 128, (q + 1) * 128)
                                cnt = 0
                                pieces = (("hi", "hi"), ("lo", "hi"),
                                          ("hi", "lo"))
                                nmm = len(pieces) * 2 * KT
                                for hp, wp in pieces:
                                    for di, dr in enumerate(("f", "b")):
                                        for k in range(KT):
                                            cnt += 1
                                            nc.tensor.matmul(
                                                po[:],
                                                hist[dr, hp][:, k, bi, qsl],
                                                pwT[ls, wp][:, di * KT + k,
                                                            :],
                                                start=(cnt == 1),
                                                stop=(cnt == nmm))
                                nc.vector.tensor_scalar(
                                    proj[:, q, bi:bi + 1], po[:],
                                    cons_t[:, pbi:pbi + 1], None, op0=OP.add)

                    if ls == "dur":
                        nc.sync.dma_start(dram["dur_out"][:], dT[:])
                        # d rows (free layout at partitions 32b) for rng xg
                        with tc.tile_pool(name="dfp", bufs=4,
                                          space="PSUM") as dpsum:
                            for bi in range(BC):
                                po = dpsum.tile([128, n], F32, tag="df")
                                pb = 32 * bi
                                cnt = 0
                                pieces = (("hi", "hi"), ("lo", "hi"),
                                          ("hi", "lo"))
                                nmm = len(pieces) * 2 * KT
                                for hp, wp in pieces:
                                    for di, dr in enumerate(("f", "b")):
                                        for k in range(KT):
                                            cnt += 1
                                            nc.tensor.matmul(
                                                po[pb:pb + 1, :],
                                                pwT[ls, wp][:, di * KT + k,
                                                            :],
                                                hist[dr, hp][:, k, bi, :],
                                                start=(cnt == 1),
                                                stop=(cnt == nmm),
                                                tile_position=(0, pb))
                                nc.vector.tensor_scalar(
                                    dfree[pb:pb + 1, :], po[pb:pb + 1, :],
                                    cons_t[pb:pb + 1, 0:1], None, op0=OP.add)
                                nc.vector.tensor_copy(
                                    scanrhs_hi[pb:pb + 1, :],
                                    dfree[pb:pb + 1, :])
                                nc.vector.tensor_sub(
                                    scanrhs_lo[pb:pb + 1, :],
                                    dfree[pb:pb + 1, :],
                                    scanrhs_hi[pb:pb + 1, :])
                        # centers: -c^T = -(L - 0.5I) @ d
                        with tc.tile_pool(name="ctr", bufs=4,
                                          space="PSUM") as cpsum:
                            for q in range(nch):
                                po = cpsum.tile([128, BC], F32, tag="cp")
                                for s in range(q + 1):
                                    nc.tensor.matmul(
                                        po[:],
                                        ltri[:, 1 if s == q else 0, :],
                                        dT[:, s, :],
                                        start=(s == 0), stop=(s == q))
                                nc.vector.tensor_scalar(
                                    negcT[:, q, :], po[:], -1.0, None,
                                    op0=OP.mult)

            # nir2T = -(1/r)^2
            nc.vector.reciprocal(rT[:], rT[:])
            nc.vector.scalar_tensor_tensor(nir2T[:], rT[:], -1.0, rT[:],
                                           op0=OP.mult, op1=OP.mult)

            # ================= scores + bmm =================
            nc.enter_named_scope("scores", False)
            with tc.tile_pool(name="sc", bufs=1) as scg:
                pe_t = scg.tile([128, tch, D], F32)
                nc.sync.dma_start(pe_t[:], dram["pe"][:])
                xhat_hi = scg.tile([128, nch, BC, D + 1], BF16)
                xhat_lo = scg.tile([128, nch, BC, D + 1], BF16)
                nc.sync.dma_start(xhat_hi[:], dram["xhat_hi"][:])
                nc.sync.dma_start(xhat_lo[:], dram["xhat_lo"][:])
                tiota = scg.tile([128, t_out], F32)
                nc.gpsimd.iota(tiota[:], pattern=[[1, t_out]], base=0,
                               channel_multiplier=0,
                               allow_small_or_imprecise_dtypes=True)
                with tc.tile_pool(name="wbuf", bufs=2) as wbp, \
                     tc.tile_pool(name="wtmp", bufs=2) as wtp, \
                     tc.tile_pool(name="scp", bufs=4, space="PSUM") as apsum:
                    for bi in range(BC):
                        whi = wbp.tile([128, nch, t_out], BF16, tag="whi")
                        wlo = wbp.tile([128, nch, t_out], BF16, tag="wlo")
                        for q in range(nch):
                            u = wtp.tile([128, t_out], F32, tag="u")
                            nc.vector.tensor_scalar(
                                u[:], tiota[:], negcT[:, q, bi:bi + 1], None,
                                op0=OP.add)
                            u2 = wtp.tile([128, t_out], F32, tag="u2")
                            nc.vector.scalar_tensor_tensor(
                                u2[:], u[:], nir2T[:, q, bi:bi + 1], u[:],
                                op0=OP.mult, op1=OP.mult)
                            wf = wtp.tile([128, t_out], F32, tag="wf")
                            nc.scalar.activation(wf[:], u2[:], AF.Exp)
                            nc.scalar.copy(whi[:, q, :], wf[:])
                            nc.vector.tensor_sub(wlo[:, q, :], wf[:],
                                                 whi[:, q, :])
                        nmm = 3 * nch
                        for j in range(tch):
                            po = apsum.tile([128, D + 1], F32, tag="ap")
                            jsl = slice(j * 128, (j + 1) * 128)
                            cnt = 0
                            for wp, xp in ((whi, xhat_hi), (wlo, xhat_hi),
                                           (whi, xhat_lo)):
                                for q in range(nch):
                                    cnt += 1
                                    nc.tensor.matmul(
                                        po[:], wp[:, q, jsl],
                                        xp[:, q, bi, :],
                                        start=(cnt == 1), stop=(cnt == nmm))
                            srec = wtp.tile([128, 1], F32, tag="srec")
                            nc.vector.tensor_scalar(
                                srec[:], po[:, D:D + 1], cons_t[:, 2:3],
                                None, op0=OP.add)
                            nc.vector.reciprocal(srec[:], srec[:])
                            att_t = wtp.tile([128, D], F32, tag="att")
                            nc.vector.scalar_tensor_tensor(
                                att_t[:], po[:, 0:D], srec[:, 0:1],
                                pe_t[:, j, :], op0=OP.mult, op1=OP.add)
                            nc.sync.dma_start(
                                dram["att_out"][bi, j, :, :], att_t[:])
            nc._state.pop_named_scope("scores")

    _split_excess_waits(nc)
    return nc, dram


# ---------------------------------------------------------------- host prep
def _bfsplit(a):
    hi = a.astype(ml_dtypes.bfloat16)
    lo = (a - hi.astype(np.float32)).astype(ml_dtypes.bfloat16)
    return hi, lo


def _perm4h():
    """gate order [i,f,g,o] (torch) -> chunk blocks [i,f,o,g]."""
    i = np.arange(H)
    return np.concatenate([i, H + i, 3 * H + i, 2 * H + i])


def _prep_kxm(Wt):
    """[K_total, M] -> [128, KT, M] (K on partitions)."""
    ktot, m = Wt.shape
    return Wt.reshape(ktot // 128, 128, m).transpose(1, 0, 2).copy()


def kernel(embeddings, input_lengths, T_out,
           dur_Wf, dur_Uf, dur_bf, dur_Wb, dur_Ub, dur_bb, dur_pw, dur_pb,
           rng_Wf, rng_Uf, rng_bf, rng_Wb, rng_Ub, rng_bb, rng_pw, rng_pb,
           pe, _cfg=None, _trace=False):
    global LAST_RES
    cfg = {"N": N, "T": T}
    if _cfg:
        cfg.update(_cfg)
    n, t_out = cfg["N"], cfg["T"]
    nch = n // 128
    tch = t_out // 128

    emb = np.asarray(embeddings, dtype=np.float32)
    perm = _perm4h()

    key = ("nc", n, t_out)
    if key not in _BUILD_CACHE:
        _BUILD_CACHE[key] = build_nc(cfg)
    nc, dram = _BUILD_CACHE[key]

    rep = {}
    for ls, Wf_, Uf_, bf_, Wb_, Ub_, bb_ in (
            ("dur", dur_Wf, dur_Uf, dur_bf, dur_Wb, dur_Ub, dur_bb),
            ("rng", rng_Wf, rng_Uf, rng_bf, rng_Wb, rng_Ub, rng_bb)):
        for dr, W_, U_, b_ in (("f", Wf_, Uf_, bf_), ("b", Wb_, Ub_, bb_)):
            W_ = np.asarray(W_, np.float32)
            U_ = np.asarray(U_, np.float32)
            whh = _prep_kxm(U_.T[:, perm])
            hi, lo = _bfsplit(whh)
            rep[f"whh_{ls}_{dr}_hi"], rep[f"whh_{ls}_{dr}_lo"] = hi, lo
            wx = _prep_kxm(W_[:, :D].T[:, perm])
            hi, lo = _bfsplit(wx)
            rep[f"wih_{ls}_{dr}_hi"], rep[f"wih_{ls}_{dr}_lo"] = hi, lo
            bp = np.asarray(b_, np.float32)[perm]
            bhi, blo = _bfsplit(bp)
            if ls == "dur":
                rep[f"brow_dur_{dr}"] = np.stack([bhi, blo])
            else:
                wd = np.asarray(W_, np.float32)[perm, D]
                wdhi, wdlo = _bfsplit(wd)
                za = np.zeros((98, 4 * H), ml_dtypes.bfloat16)
                zb = np.zeros((98, 4 * H), ml_dtypes.bfloat16)
                zc = np.zeros((98, 4 * H), ml_dtypes.bfloat16)
                for bi in range(BC):
                    za[32 * bi] = wdhi; za[32 * bi + 1] = bhi
                    zb[32 * bi] = wdlo; zb[32 * bi + 1] = blo
                    zc[32 * bi] = wdhi
                rep[f"wdA_rng_{dr}"] = za
                rep[f"wdB_rng_{dr}"] = zb
                rep[f"wdC_rng_{dr}"] = zc
    for ls, pw_ in (("dur", dur_pw), ("rng", rng_pw)):
        pw_ = np.asarray(pw_, np.float32).reshape(2 * H)
        pwT = pw_.reshape(2 * KT, 128).T.reshape(128, 2 * KT, 1).copy()
        hi, lo = _bfsplit(pwT)
        rep[f"pwT_{ls}_hi"], rep[f"pwT_{ls}_lo"] = hi, lo
    consrow = np.array([float(np.asarray(dur_pb).reshape(-1)[0]),
                        float(np.asarray(rng_pb).reshape(-1)[0]),
                        EPS, 0.0], np.float32)
    rep["cons"] = np.tile(consrow[None, :], (128, 1))
    sri = np.zeros((128, n), ml_dtypes.bfloat16)
    for bi in range(BC):
        sri[32 * bi + 1] = 1.0
    rep["scanrhs_init"] = sri
    ones_blk = np.ones((128, 128), np.float32)
    tri_blk = (np.tril(np.ones((128, 128), np.float32))
               - 0.5 * np.eye(128, dtype=np.float32))
    rep["ltri"] = np.stack([ones_blk, tri_blk.T.copy()], axis=1)
    pe_ = np.asarray(pe, np.float32)[:t_out]
    rep["pe"] = pe_.reshape(tch, 128, D).transpose(1, 0, 2).copy()

    in_maps = []
    for c in range(NCORES):
        m = dict(rep)
        ec = emb[c * BC:(c + 1) * BC, :n]             # [BC, n, D]
        xT = ec.transpose(2, 0, 1).reshape(KT, 128, BC, n)\
            .transpose(1, 0, 2, 3).copy()             # [128, KT, BC, n]
        hi, lo = _bfsplit(xT)
        m["xT_hi"], m["xT_lo"] = hi, lo
        xhat = np.concatenate(
            [ec, np.ones((BC, n, 1), np.float32)], axis=2)
        xhat = xhat.reshape(BC, nch, 128, D + 1).transpose(2, 1, 0, 3).copy()
        hi, lo = _bfsplit(xhat)
        m["xhat_hi"], m["xhat_lo"] = hi, lo
        in_maps.append(m)

    if _trace:
        import trnprof
        trnprof.install()
    res = run_bass_kernel_spmd(nc, in_maps, core_ids=list(range(NCORES)),
                               trace=_trace)
    LAST_RES = res

    durations = np.zeros((B, n, 1), np.float32)
    att = np.zeros((B, t_out, D), np.float32)
    for c in range(NCORES):
        r = res.results[c]
        durations[c * BC:(c + 1) * BC, :, 0] = \
            r["dur_out"].transpose(2, 1, 0).reshape(BC, n)
        att[c * BC:(c + 1) * BC] = r["att_out"].reshape(BC, t_out, D)
    return durations, att


# revision 14
# speedup vs baseline: 1.0644x; 1.0155x over previous
"""Trainium2 Bass kernel for Gaussian-upsampling attention (duration/range
BiLSTM predictors + Gaussian score attention), data-parallel over batch
across 8 NeuronCores.

kernel(**inputs) takes the full unsharded inputs (as in reference
setup_inputs) and returns (durations [B,N,1] f32, att [B,T,D] f32).

Numerics: all matmuls run as bf16 hi/lo-split pieces (weights AND moving
operands split into bf16 high + bf16 residual; three cross products
accumulate in fp32 PSUM), giving ~1.5e-5 effective relative error, except
the small cumsum (triangular) matmul which is plain fp32. Activations
(sigmoid/tanh/exp) use the ACT LUTs (~1e-6).
"""
import sys

for _p in ("/opt/trn_rl_repo", "/root/.axon_site", "/root/.axon_site/_ro/trn_rl_repo"):
    if _p not in sys.path:
        sys.path.append(_p)

import numpy as np
import ml_dtypes

import concourse.bass as bass
import concourse.mybir as mybir
import concourse.tile as tile
import bass_rust
from concourse.bass_utils import run_bass_kernel_spmd

F32 = mybir.dt.float32
BF16 = mybir.dt.bfloat16
AF = mybir.ActivationFunctionType
OP = mybir.AluOpType

# problem shapes (hardcoded per spec)
B, N, D, H, T = 32, 512, 256, 256, 2048
NCORES = 8
BC = B // NCORES          # batches per core = 4
KT = H // 128             # K-tiles of hidden dim = 2
MC = 4 * H // 128         # gate chunks = 8
EPS = 1e-6
XQ = 128                  # xg staging chunk (tokens)

_BUILD_CACHE = {}
LAST_RES = None


# ---------------------------------------------------------------- wait split
def _split_excess_waits(nc, cap=1):
    """walrus in this env rejects >cap sync-waits on an instruction; hoist
    excess waits onto preceding same-engine NOPs."""
    n_created = 0
    for f in nc.m.functions:
        for blk in f.blocks:
            insts = blk.instructions
            i = 0
            while i < len(insts):
                inst = insts[i]
                si = inst.sync_info
                waits = list(si.on_wait) if si is not None else []
                if len(waits) > cap:
                    keep = waits[:cap]
                    extra = waits[cap:]
                    inst.sync_info = bass_rust.SyncInfo(
                        on_wait=keep, on_update=list(si.on_update))
                    pos = i
                    for j in range(0, len(extra), cap):
                        chunk = extra[j:j + cap]
                        nop = mybir.InstNoOp(
                            name=f"I-waitsplit-{n_created}", ins=[], outs=[])
                        nop.engine = inst.engine
                        nop.sync_info = bass_rust.SyncInfo(
                            on_wait=chunk, on_update=[])
                        nc.register_instruction(nop)
                        insts.insert(pos, nop)
                        pos += 1
                        i += 1
                        n_created += 1
                i += 1
    return n_created


# ---------------------------------------------------------------- build
def build_nc(cfg):
    n = cfg["N"]; t_out = cfg["T"]
    nch = n // 128
    tch = t_out // 128
    nq = n // XQ
    nc = bass.Bass()

    dram = {}

    def din(name, shape, dtype=F32):
        dram[name] = nc.declare_dram_parameter(name, list(shape), dtype,
                                               isOutput=False)
        return dram[name]

    def dout(name, shape, dtype=F32):
        dram[name] = nc.declare_dram_parameter(name, list(shape), dtype,
                                               isOutput=True)
        return dram[name]

    din("xT_hi", [128, KT, BC, n], BF16)
    din("xT_lo", [128, KT, BC, n], BF16)
    din("xhat_hi", [128, nch, BC, D + 1], BF16)
    din("xhat_lo", [128, nch, BC, D + 1], BF16)
    for ls in ("dur", "rng"):
        for dr in ("f", "b"):
            for pc in ("hi", "lo"):
                din(f"whh_{ls}_{dr}_{pc}", [128, KT, 4 * H], BF16)
                din(f"wih_{ls}_{dr}_{pc}", [128, KT, 4 * H], BF16)
    for dr in ("f", "b"):
        din(f"brow_dur_{dr}", [2, 4 * H], BF16)    # [bias_hi; bias_lo]
        din(f"wdA_rng_{dr}", [98, 4 * H], BF16)    # [w_d_hi; bias_hi] @ 32b
        din(f"wdB_rng_{dr}", [98, 4 * H], BF16)    # [w_d_lo; bias_lo]
        din(f"wdC_rng_{dr}", [98, 4 * H], BF16)    # [w_d_hi; 0]
    for ls in ("dur", "rng"):
        for pc in ("hi", "lo"):
            din(f"pwT_{ls}_{pc}", [128, 2 * KT, 1], BF16)
    din("cons", [128, 4], F32)          # cols: dur_pb, rng_pb, eps, 0
    din("scanrhs_init", [128, n], BF16)  # ones rows at partitions 32b+1
    din("ltri", [128, 2, 128], F32)     # [ones block, (tril-0.5I)^T block]
    din("pe", [128, tch, D], F32)

    dout("dur_out", [128, nch, BC], F32)
    dout("att_out", [BC, tch, 128, D], F32)
    if cfg.get("DEBUG"):
        dout("dbg_xgf", [128, MC, BC, XQ], F32)
        dout("dbg_histfhi", [128, KT, BC, n], BF16)
        dout("dbg_histflo", [128, KT, BC, n], BF16)
        dout("dbg_histbhi", [128, KT, BC, n], BF16)
        dout("dbg_gs1", [128, 2, MC, BC], F32)

    with tile.TileContext(nc) as tc:
        with tc.tile_pool(name="glob", bufs=1) as glob:
            cons_t = glob.tile([128, 4], F32)
            nc.sync.dma_start(cons_t[:], dram["cons"][:])
            ltri = glob.tile([128, 2, 128], F32)
            nc.sync.dma_start(ltri[:], dram["ltri"][:])
            pwT = {}
            for ls in ("dur", "rng"):
                for pc in ("hi", "lo"):
                    pwT[ls, pc] = glob.tile([128, 2 * KT, 1], BF16, name=f"pwT_{ls}_{pc}",
                                            tag=f"pwT_{ls}_{pc}")
                    nc.sync.dma_start(pwT[ls, pc][:], dram[f"pwT_{ls}_{pc}"][:])
            dT = glob.tile([128, nch, BC], F32)
            negcT = glob.tile([128, nch, BC], F32)
            rT = glob.tile([128, nch, BC], F32)
            nir2T = glob.tile([128, nch, BC], F32)
            # rng-xg rhs rows: [d_hi;1] and [d_lo;0] at partitions {32b,32b+1}
            scanrhs_hi = glob.tile([128, n], BF16)
            scanrhs_lo = glob.tile([128, n], BF16)
            nc.sync.dma_start(scanrhs_hi[:], dram["scanrhs_init"][:])
            nc.vector.memset(scanrhs_lo[:], 0.0)
            dfree = glob.tile([128, n], F32)
            zero_h = glob.tile([128, KT, BC], BF16)
            nc.vector.memset(zero_h[:], 0.0)
            ones2 = glob.tile([2, n], BF16)
            nc.vector.memset(ones2[:], 1.0)

            # ================= LSTM phases =================
            for ls in ("dur", "rng"):
                with tc.tile_pool(name=f"ph{ls}", bufs=1) as php:
                    whh = {}; wih = {}
                    for dr in ("f", "b"):
                        for pc in ("hi", "lo"):
                            whh[dr, pc] = php.tile([128, KT, 4 * H], BF16, name=f"whh{dr}{pc}",
                                                   tag=f"whh{dr}{pc}")
                            nc.sync.dma_start(whh[dr, pc][:],
                                              dram[f"whh_{ls}_{dr}_{pc}"][:])
                            wih[dr, pc] = php.tile([128, KT, 4 * H], BF16, name=f"wih{dr}{pc}",
                                                   tag=f"wih{dr}{pc}")
                            nc.sync.dma_start(wih[dr, pc][:],
                                              dram[f"wih_{ls}_{dr}_{pc}"][:])
                    xT_hi = php.tile([128, KT, BC, n], BF16, tag="xthi")
                    xT_lo = php.tile([128, KT, BC, n], BF16, tag="xtlo")
                    nc.sync.dma_start(xT_hi[:], dram["xT_hi"][:])
                    nc.sync.dma_start(xT_lo[:], dram["xT_lo"][:])
                    wrows = {}
                    for dr in ("f", "b"):
                        if ls == "dur":
                            br = php.tile([2, 4 * H], BF16, tag=f"br{dr}")
                            nc.sync.dma_start(br[:], dram[f"brow_dur_{dr}"][:])
                            wrows[dr] = br
                        else:
                            rows = []
                            for nm in ("wdA", "wdB", "wdC"):
                                wt = php.tile([98, 4 * H], BF16, name=f"{nm}{dr}",
                                              tag=f"{nm}{dr}")
                                nc.sync.dma_start(
                                    wt[:], dram[f"{nm}_rng_{dr}"][:])
                                rows.append(wt)
                            wrows[dr] = rows

                    hist = {}
                    for dr in ("f", "b"):
                        for pc in ("hi", "lo"):
                            hist[dr, pc] = php.tile(
                                [128, KT, BC, n], BF16,
                                name=f"hist{dr}{pc}", tag=f"hist{dr}{pc}")
                    c_t = {}
                    for dr in ("f", "b"):
                        c_t[dr] = php.tile([128, KT, BC], F32, name=f"c{dr}",
                                           tag=f"c{dr}")
                        nc.vector.memset(c_t[dr][:], 0.0)

                    # ---- xg staging GEMM (one XQ-token chunk, one dir).
                    # Returns (tile, [block emitters]) so blocks can be
                    # spread between scan steps to fill PE stalls. ----
                    def make_xg_chunk(dr, q, xgpool, psum):
                        xt = xgpool.tile([128, MC, BC, XQ], F32,
                                         name=f"xg{dr}", tag=f"xg{dr}")
                        tsl = slice(q * XQ, (q + 1) * XQ)

                        def mk(m, bi):
                            def emit():
                                msl = slice(m * 128, (m + 1) * 128)
                                po = psum.tile([128, XQ], F32, name="xp",
                                               tag="xp")
                                nmm = 3 * KT + (1 if ls == "dur" else 3)
                                cnt = 0
                                for wp, xp in ((wih[dr, "hi"], xT_hi),
                                               (wih[dr, "lo"], xT_hi),
                                               (wih[dr, "hi"], xT_lo)):
                                    for k in range(KT):
                                        cnt += 1
                                        nc.tensor.matmul(
                                            po[:], wp[:, k, msl],
                                            xp[:, k, bi, tsl],
                                            start=(cnt == 1),
                                            stop=(cnt == nmm))
                                if ls == "dur":
                                    cnt += 1
                                    nc.tensor.matmul(
                                        po[:], wrows[dr][0:2, msl],
                                        ones2[0:2, tsl],
                                        start=False, stop=(cnt == nmm))
                                else:
                                    pb = 32 * bi
                                    for wt, rr in (
                                            (wrows[dr][0], scanrhs_hi),
                                            (wrows[dr][1], scanrhs_hi),
                                            (wrows[dr][2], scanrhs_lo)):
                                        cnt += 1
                                        nc.tensor.matmul(
                                            po[:],
                                            wt[pb:pb + 2, msl],
                                            rr[pb:pb + 2, tsl],
                                            start=False, stop=(cnt == nmm),
                                            tile_position=(pb, 0))
                                nc.scalar.copy(xt[:, m, bi, :], po[:])
                            return emit

                        blocks = [mk(m, bi) for m in range(MC)
                                  for bi in range(BC)]
                        return xt, blocks

                    def emit_xg_chunk(dr, q, xgpool, psum):
                        xt, blocks = make_xg_chunk(dr, q, xgpool, psum)
                        for blk in blocks:
                            blk()
                        return xt

                    # ---- the scan ----
                    nc.enter_named_scope(f"scan_{ls}", False)
                    with tc.tile_pool(name=f"xgq{ls}", bufs=2) as xgpool, \
                         tc.tile_pool(name=f"xgp{ls}", bufs=3,
                                      space="PSUM") as xpsum, \
                         tc.tile_pool(name=f"scan{ls}", bufs=3) as scp, \
                         tc.tile_pool(name=f"scanp{ls}", bufs=2,
                                      space="PSUM") as spsum:
                        xq_cur = {"f": emit_xg_chunk("f", 0, xgpool, xpsum),
                                  "b": emit_xg_chunk("b", nq - 1, xgpool,
                                                     xpsum)}
                        if cfg.get("DEBUG") and ls == "dur":
                            nc.sync.dma_start(dram["dbg_xgf"][:],
                                              xq_cur["f"][:])
                        xq_nxt = {}
                        pending_blocks = []
                        for t in range(n):
                            qw = t // XQ
                            if t % XQ == 8 and qw + 1 < nq:
                                xq_nxt["f"], bl_f = make_xg_chunk(
                                    "f", qw + 1, xgpool, xpsum)
                                xq_nxt["b"], bl_b = make_xg_chunk(
                                    "b", nq - 2 - qw, xgpool, xpsum)
                                # interleave f/b blocks
                                pending_blocks = [blk for pair in
                                                  zip(bl_f, bl_b)
                                                  for blk in pair]
                            if pending_blocks and t % XQ >= 8:
                                pending_blocks.pop(0)()
                            if t % XQ == 0 and t > 0:
                                for blk in pending_blocks:
                                    blk()
                                pending_blocks = []
                                xq_cur = dict(xq_nxt)
                            toks = {"f": t, "b": n - 1 - t}
                            for di, dr in enumerate(("f", "b")):
                                tok = toks[dr]
                                po = spsum.tile([128, MC * BC], F32,
                                                tag=f"g{dr}")
                                if t == 0:
                                    pieces = [(whh[dr, "hi"], zero_h, None),
                                              (whh[dr, "lo"], zero_h, None),
                                              (whh[dr, "hi"], zero_h, None)]
                                else:
                                    prev = tok + (1 if dr == "b" else -1)
                                    pieces = [
                                        (whh[dr, "hi"], hist[dr, "hi"], prev),
                                        (whh[dr, "lo"], hist[dr, "hi"], prev),
                                        (whh[dr, "hi"], hist[dr, "lo"], prev)]
                                for m in range(MC):
                                    cnt = 0
                                    for wp, hp, prev in pieces:
                                        for k in range(KT):
                                            cnt += 1
                                            rhs = (hp[:, k, :] if prev is None
                                                   else hp[:, k, :, prev])
                                            nc.tensor.matmul(
                                                po[:, m * BC:(m + 1) * BC],
                                                wp[:, k,
                                                   m * 128:(m + 1) * 128],
                                                rhs,
                                                start=(cnt == 1),
                                                stop=(cnt == 3 * KT))
                                xgt = xq_cur[dr]
                                off = tok % XQ
                                gs = scp.tile([128, MC, BC], F32,
                                              name=f"gs{dr}", tag=f"gs{dr}")
                                hp_ctx = tc.high_priority(offset=300)
                                hp_ctx.__enter__()
                                nc.vector.scalar_tensor_tensor(
                                    gs[:],
                                    po[:].rearrange("p (m b) -> p m b", b=BC),
                                    0.0, xgt[:, :, :, off],
                                    op0=OP.add, op1=OP.add)
                                sg = scp.tile([128, MC, BC], F32,
                                              name=f"sg{dr}", tag=f"sg{dr}")
                                nc.scalar.activation(sg[:, 0:6, :],
                                                     gs[:, 0:6, :],
                                                     AF.Sigmoid)
                                nc.scalar.activation(sg[:, 6:8, :],
                                                     gs[:, 6:8, :], AF.Tanh)
                                m1 = scp.tile([128, KT, BC], F32,
                                              name=f"m1{dr}", tag=f"m1{dr}")
                                nc.vector.tensor_mul(m1[:], sg[:, 0:2, :],
                                                     sg[:, 6:8, :])
                                nc.vector.tensor_mul(c_t[dr][:],
                                                     sg[:, 2:4, :],
                                                     c_t[dr][:])
                                nc.vector.tensor_add(c_t[dr][:], c_t[dr][:],
                                                     m1[:])
                                th = scp.tile([128, KT, BC], F32,
                                              name=f"th{dr}", tag=f"th{dr}")
                                nc.scalar.activation(th[:], c_t[dr][:],
                                                     AF.Tanh)
                                hfp = scp.tile([128, KT, BC], F32,
                                               name=f"hfp{dr}",
                                               tag=f"hfp{dr}")
                                nc.vector.tensor_mul(hfp[:], sg[:, 4:6, :],
                                                     th[:])
                                nc.vector.tensor_copy(
                                    hist[dr, "hi"][:, :, :, tok], hfp[:])
                                nc.vector.tensor_sub(
                                    hist[dr, "lo"][:, :, :, tok],
                                    hfp[:], hist[dr, "hi"][:, :, :, tok])
                                hp_ctx.__exit__(None, None, None)

                    if cfg.get("DEBUG") and ls == "dur":
                        nc.sync.dma_start(dram["dbg_histfhi"][:],
                                          hist["f", "hi"][:])
                        nc.sync.dma_start(dram["dbg_histflo"][:],
                                          hist["f", "lo"][:])
                        nc.sync.dma_start(dram["dbg_histbhi"][:],
                                          hist["b", "hi"][:])
                    nc._state.pop_named_scope(f"scan_{ls}")
                    # ---- projection to d^T (dur) or r^T (rng), += pb ----
                    proj = dT if ls == "dur" else rT
                    pbi = 0 if ls == "dur" else 1
                    with tc.tile_pool(name=f"pj{ls}", bufs=4,
                                      space="PSUM") as ppsum:
                        for bi in range(BC):
                            for q in range(nch):
                                po = ppsum.tile([128, 1], F32, tag="pp")
                                qsl = slice(q * 128, (q + 1) * 128)
                                cnt = 0
                                pieces = (("hi", "hi"), ("lo", "hi"),
                                          ("hi", "lo"))
                                nmm = len(pieces) * 2 * KT
                                for hp, wp in pieces:
                                    for di, dr in enumerate(("f", "b")):
                                        for k in range(KT):
                                            cnt += 1
                                            nc.tensor.matmul(
                                                po[:],
                                                hist[dr, hp][:, k, bi, qsl],
                                                pwT[ls, wp][:, di * KT + k,
                                                            :],
                                                start=(cnt == 1),
                                                stop=(cnt == nmm))
                                nc.vector.tensor_scalar(
                                    proj[:, q, bi:bi + 1], po[:],
                                    cons_t[:, pbi:pbi + 1], None, op0=OP.add)

                    if ls == "dur":
                        nc.sync.dma_start(dram["dur_out"][:], dT[:])
                        # d rows (free layout at partitions 32b) for rng xg
                        with tc.tile_pool(name="dfp", bufs=4,
                                          space="PSUM") as dpsum:
                            for bi in range(BC):
                                po = dpsum.tile([128, n], F32, tag="df")
                                pb = 32 * bi
                                cnt = 0
                                pieces = (("hi", "hi"), ("lo", "hi"),
                                          ("hi", "lo"))
                                nmm = len(pieces) * 2 * KT
                                for hp, wp in pieces:
                                    for di, dr in enumerate(("f", "b")):
                                        for k in range(KT):
                                            cnt += 1
                                            nc.tensor.matmul(
                                                po[pb:pb + 1, :],
                                                pwT[ls, wp][:, di * KT + k,
                                                            :],
                                                hist[dr, hp][:, k, bi, :],
                                                start=(cnt == 1),
                                                stop=(cnt == nmm),
                                                tile_position=(0, pb))
                                nc.vector.tensor_scalar(
                                    dfree[pb:pb + 1, :], po[pb:pb + 1, :],
                                    cons_t[pb:pb + 1, 0:1], None, op0=OP.add)
                                nc.vector.tensor_copy(
                                    scanrhs_hi[pb:pb + 1, :],
                                    dfree[pb:pb + 1, :])
                                nc.vector.tensor_sub(
                                    scanrhs_lo[pb:pb + 1, :],
                                    dfree[pb:pb + 1, :],
                                    scanrhs_hi[pb:pb + 1, :])
                        # centers: -c^T = -(L - 0.5I) @ d
                        with tc.tile_pool(name="ctr", bufs=4,
                                          space="PSUM") as cpsum:
                            for q in range(nch):
                                po = cpsum.tile([128, BC], F32, tag="cp")
                                for s in range(q + 1):
                                    nc.tensor.matmul(
                                        po[:],
                                        ltri[:, 1 if s == q else 0, :],
                                        dT[:, s, :],
                                        start=(s == 0), stop=(s == q))
                                nc.vector.tensor_scalar(
                                    negcT[:, q, :], po[:], -1.0, None,
                                    op0=OP.mult)

            # nir2T = -(1/r)^2
            nc.vector.reciprocal(rT[:], rT[:])
            nc.vector.scalar_tensor_tensor(nir2T[:], rT[:], -1.0, rT[:],
                                           op0=OP.mult, op1=OP.mult)

            # ================= scores + bmm =================
            nc.enter_named_scope("scores", False)
            with tc.tile_pool(name="sc", bufs=1) as scg:
                pe_t = scg.tile([128, tch, D], F32)
                nc.sync.dma_start(pe_t[:], dram["pe"][:])
                xhat_hi = scg.tile([128, nch, BC, D + 1], BF16)
                xhat_lo = scg.tile([128, nch, BC, D + 1], BF16)
                nc.sync.dma_start(xhat_hi[:], dram["xhat_hi"][:])
                nc.sync.dma_start(xhat_lo[:], dram["xhat_lo"][:])
                tiota = scg.tile([128, t_out], F32)
                nc.gpsimd.iota(tiota[:], pattern=[[1, t_out]], base=0,
                               channel_multiplier=0,
                               allow_small_or_imprecise_dtypes=True)
                with tc.tile_pool(name="wbuf", bufs=2) as wbp, \
                     tc.tile_pool(name="wtmp", bufs=2) as wtp, \
                     tc.tile_pool(name="scp", bufs=4, space="PSUM") as apsum:
                    for bi in range(BC):
                        whi = wbp.tile([128, nch, t_out], BF16, tag="whi")
                        wlo = wbp.tile([128, nch, t_out], BF16, tag="wlo")
                        for q in range(nch):
                            u = wtp.tile([128, t_out], F32, tag="u")
                            nc.vector.tensor_scalar(
                                u[:], tiota[:], negcT[:, q, bi:bi + 1], None,
                                op0=OP.add)
                            u2 = wtp.tile([128, t_out], F32, tag="u2")
                            nc.vector.scalar_tensor_tensor(
                                u2[:], u[:], nir2T[:, q, bi:bi + 1], u[:],
                                op0=OP.mult, op1=OP.mult)
                            wf = wtp.tile([128, t_out], F32, tag="wf")
                            nc.scalar.activation(wf[:], u2[:], AF.Exp)
                            nc.scalar.copy(whi[:, q, :], wf[:])
                            nc.vector.tensor_sub(wlo[:, q, :], wf[:],
                                                 whi[:, q, :])
                        nmm = 3 * nch
                        for j in range(tch):
                            po = apsum.tile([128, D + 1], F32, tag="ap")
                            jsl = slice(j * 128, (j + 1) * 128)
                            cnt = 0
                            for wp, xp in ((whi, xhat_hi), (wlo, xhat_hi),
                                           (whi, xhat_lo)):
                                for q in range(nch):
                                    cnt += 1
                                    nc.tensor.matmul(
                                        po[:], wp[:, q, jsl],
                                        xp[:, q, bi, :],
                                        start=(cnt == 1), stop=(cnt == nmm))
                            srec = wtp.tile([128, 1], F32, tag="srec")
                            nc.vector.tensor_scalar(
                                srec[:], po[:, D:D + 1], cons_t[:, 2:3],
                                None, op0=OP.add)
                            nc.vector.reciprocal(srec[:], srec[:])
                            att_t = wtp.tile([128, D], F32, tag="att")
                            nc.vector.scalar_tensor_tensor(
                                att_t[:], po[:, 0:D], srec[:, 0:1],
                                pe_t[:, j, :], op0=OP.mult, op1=OP.add)
                            nc.sync.dma_start(
                                dram["att_out"][bi, j, :, :], att_t[:])
            nc._state.pop_named_scope("scores")

    _split_excess_waits(nc)
    return nc, dram


# ---------------------------------------------------------------- host prep
def _bfsplit(a):
    hi = a.astype(ml_dtypes.bfloat16)
    lo = (a - hi.astype(np.float32)).astype(ml_dtypes.bfloat16)
    return hi, lo


def _perm4h():
    """gate order [i,f,g,o] (torch) -> chunk blocks [i,f,o,g]."""
    i = np.arange(H)
    return np.concatenate([i, H + i, 3 * H + i, 2 * H + i])


def _prep_kxm(Wt):
    """[K_total, M] -> [128, KT, M] (K on partitions)."""
    ktot, m = Wt.shape
    return Wt.reshape(ktot // 128, 128, m).transpose(1, 0, 2).copy()


def kernel(embeddings, input_lengths, T_out,
           dur_Wf, dur_Uf, dur_bf, dur_Wb, dur_Ub, dur_bb, dur_pw, dur_pb,
           rng_Wf, rng_Uf, rng_bf, rng_Wb, rng_Ub, rng_bb, rng_pw, rng_pb,
           pe, _cfg=None, _trace=False):
    global LAST_RES
    cfg = {"N": N, "T": T}
    if _cfg:
        cfg.update(_cfg)
    n, t_out = cfg["N"], cfg["T"]
    nch = n // 128
    tch = t_out // 128

    emb = np.asarray(embeddings, dtype=np.float32)
    perm = _perm4h()

    key = ("nc", n, t_out)
    if key not in _BUILD_CACHE:
        _BUILD_CACHE[key] = build_nc(cfg)
    nc, dram = _BUILD_CACHE[key]

    rep = {}
    for ls, Wf_, Uf_, bf_, Wb_, Ub_, bb_ in (
            ("dur", dur_Wf, dur_Uf, dur_bf, dur_Wb, dur_Ub, dur_bb),
            ("rng", rng_Wf, rng_Uf, rng_bf, rng_Wb, rng_Ub, rng_bb)):
        for dr, W_, U_, b_ in (("f", Wf_, Uf_, bf_), ("b", Wb_, Ub_, bb_)):
            W_ = np.asarray(W_, np.float32)
            U_ = np.asarray(U_, np.float32)
            whh = _prep_kxm(U_.T[:, perm])
            hi, lo = _bfsplit(whh)
            rep[f"whh_{ls}_{dr}_hi"], rep[f"whh_{ls}_{dr}_lo"] = hi, lo
            wx = _prep_kxm(W_[:, :D].T[:, perm])
            hi, lo = _bfsplit(wx)
            rep[f"wih_{ls}_{dr}_hi"], rep[f"wih_{ls}_{dr}_lo"] = hi, lo
            bp = np.asarray(b_, np.float32)[perm]
            bhi, blo = _bfsplit(bp)
            if ls == "dur":
                rep[f"brow_dur_{dr}"] = np.stack([bhi, blo])
            else:
                wd = np.asarray(W_, np.float32)[perm, D]
                wdhi, wdlo = _bfsplit(wd)
                za = np.zeros((98, 4 * H), ml_dtypes.bfloat16)
                zb = np.zeros((98, 4 * H), ml_dtypes.bfloat16)
                zc = np.zeros((98, 4 * H), ml_dtypes.bfloat16)
                for bi in range(BC):
                    za[32 * bi] = wdhi; za[32 * bi + 1] = bhi
                    zb[32 * bi] = wdlo; zb[32 * bi + 1] = blo
                    zc[32 * bi] = wdhi
                rep[f"wdA_rng_{dr}"] = za
                rep[f"wdB_rng_{dr}"] = zb
                rep[f"wdC_rng_{dr}"] = zc
    for ls, pw_ in (("dur", dur_pw), ("rng", rng_pw)):
        pw_ = np.asarray(pw_, np.float32).reshape(2 * H)
        pwT = pw_.reshape(2 * KT, 128).T.reshape(128, 2 * KT, 1).copy()
        hi, lo = _bfsplit(pwT)
        rep[f"pwT_{ls}_hi"], rep[f"pwT_{ls}_lo"] = hi, lo
    consrow = np.array([float(np.asarray(dur_pb).reshape(-1)[0]),
                        float(np.asarray(rng_pb).reshape(-1)[0]),
                        EPS, 0.0], np.float32)
    rep["cons"] = np.tile(consrow[None, :], (128, 1))
    sri = np.zeros((128, n), ml_dtypes.bfloat16)
    for bi in range(BC):
        sri[32 * bi + 1] = 1.0
    rep["scanrhs_init"] = sri
    ones_blk = np.ones((128, 128), np.float32)
    tri_blk = (np.tril(np.ones((128, 128), np.float32))
               - 0.5 * np.eye(128, dtype=np.float32))
    rep["ltri"] = np.stack([ones_blk, tri_blk.T.copy()], axis=1)
    pe_ = np.asarray(pe, np.float32)[:t_out]
    rep["pe"] = pe_.reshape(tch, 128, D).transpose(1, 0, 2).copy()

    in_maps = []
    for c in range(NCORES):
        m = dict(rep)
        ec = emb[c * BC:(c + 1) * BC, :n]             # [BC, n, D]
        xT = ec.transpose(2, 0, 1).reshape(KT, 128, BC, n)\
            .transpose(1, 0, 2, 3).copy()             # [128, KT, BC, n]
        hi, lo = _bfsplit(xT)
        m["xT_hi"], m["xT_lo"] = hi, lo
        xhat = np.concatenate(
            [ec, np.ones((BC, n, 1), np.float32)], axis=2)
        xhat = xhat.reshape(BC, nch, 128, D + 1).transpose(2, 1, 0, 3).copy()
        hi, lo = _bfsplit(xhat)
        m["xhat_hi"], m["xhat_lo"] = hi, lo
        in_maps.append(m)

    if _trace:
        import trnprof
        trnprof.install()
    res = run_bass_kernel_spmd(nc, in_maps, core_ids=list(range(NCORES)),
                               trace=_trace)
    LAST_RES = res

    durations = np.zeros((B, n, 1), np.float32)
    att = np.zeros((B, t_out, D), np.float32)
    for c in range(NCORES):
        r = res.results[c]
        durations[c * BC:(c + 1) * BC, :, 0] = \
            r["dur_out"].transpose(2, 1, 0).reshape(BC, n)
        att[c * BC:(c + 1) * BC] = r["att_out"].reshape(BC, t_out, D)
    return durations, att
